# revision 1
# baseline (speedup 1.0000x reference)
"""Trainium2 Bass kernel for CayleyCirculantSSMLayer.

Math: lambda_j = (1-i*w_j)/(1+i*w_j) lies on the unit circle, so the causal
conv h[t] = sum_{s<=t} Re(lambda^{t-s}) Bu[s] factors through a rotated
cumulative sum:  x[t] = lam^tau * (Q_chunk + cumsum(conj(lam)^sigma Bu)),
with the carry Q chained across 256-row chunks by a per-channel rotation.
All heavy ops are matmuls (f32r) / elementwise; cumsum = triangular matmul
with the data tile as the stationary operand, which lands the state in
(channel, time) layout, ready for the output projection.

Sharding: 8 cores = 4 samples x 2 output-d-halves. Each core consumes the
full (pre-transposed) sample uT, computes Bu + conv for all 512 channels,
projects to its 512 output dims, adds the skip D*u, writes yT (512, 8192).
The d-axis of uT/bwT is permuted per-core so one SPMD program serves both
halves (contraction is permutation invariant).
"""
import sys
import numpy as np

for p in ("/opt/trn_rl_repo",):
    if p not in sys.path:
        sys.path.insert(0, p)

from concourse import bass, bacc, mybir, tile
from concourse import bass_utils

D_MODEL = 1024
STATE = 512
BATCH = 4
SEQ = 8192
L = 256                   # carry-chunk length (rows)
GL = 512                  # group length (rows) = 2 chunks
NGROUP = SEQ // GL        # 16
NT = 4                    # state n-tiles of 128
DT = mybir.dt.float32
F32R = mybir.dt.float32r

_CACHE = {}


def _r(ap):
    return ap.bitF32R


def build_nc(mm_dtype="f32r"):
    nc = bacc.Bacc(None, target_bir_lowering=False)
    uT_d = nc.dram_tensor("uT", [D_MODEL, SEQ], F32R, kind="ExternalInput")
    bwT_d = nc.dram_tensor("bwT", [D_MODEL, STATE], F32R, kind="ExternalInput")
    cwT_d = nc.dram_tensor("cwT", [STATE, STATE], F32R, kind="ExternalInput")
    c0Sa_d = nc.dram_tensor("c0Sa", [128, STATE], DT, kind="ExternalInput")
    c0Sb_d = nc.dram_tensor("c0Sb", [128, STATE], DT, kind="ExternalInput")
    ms0Sa_d = nc.dram_tensor("ms0Sa", [128, STATE], DT, kind="ExternalInput")
    ms0Sb_d = nc.dram_tensor("ms0Sb", [128, STATE], DT, kind="ExternalInput")
    c0T3_d = nc.dram_tensor("c0T3", [128, NT, GL], DT, kind="ExternalInput")
    s0T3_d = nc.dram_tensor("s0T3", [128, NT, GL], DT, kind="ExternalInput")
    cL4_d = nc.dram_tensor("cL4", [128, NT], DT, kind="ExternalInput")
    sL4_d = nc.dram_tensor("sL4", [128, NT], DT, kind="ExternalInput")
    UO_d = nc.dram_tensor("UO", [128, 256], F32R, kind="ExternalInput")
    ZU_d = nc.dram_tensor("ZU", [128, 256], F32R, kind="ExternalInput")
    yT_d = nc.dram_tensor("yT", [STATE, SEQ], DT, kind="ExternalOutput")

    
    with tile.TileContext(nc) as tc:
        with (
            tc.tile_pool(name="const", bufs=1) as cpool,
            tc.tile_pool(name="ut", bufs=3) as upool,
            tc.tile_pool(name="bus", bufs=3) as bupool,
            tc.tile_pool(name="v", bufs=3) as vpool,
            tc.tile_pool(name="agrp", bufs=2) as apool,
            tc.tile_pool(name="pgrp", bufs=2) as ppool,
            tc.tile_pool(name="hgrp", bufs=2) as hpool,
            tc.tile_pool(name="qc", bufs=3) as qpool,
            tc.tile_pool(name="yo", bufs=1) as ypool,
            tc.tile_pool(name="ps_bu", bufs=2, space="PSUM") as psbu,
            tc.tile_pool(name="ps_w", bufs=2, space="PSUM") as psw,
            tc.tile_pool(name="ps_y", bufs=2, space="PSUM") as psy,
        ):
            # ---- constants / weights resident in SBUF ----
            bwT = []
            for k in range(8):
                t = cpool.tile([128, STATE], F32R, tag=f"bw{k}")
                nc.sync.dma_start(t[:], bwT_d[k * 128:(k + 1) * 128, :])
                bwT.append(t)
            cwT = []
            for k in range(4):
                t = cpool.tile([128, STATE], F32R, tag=f"cw{k}")
                nc.sync.dma_start(t[:], cwT_d[k * 128:(k + 1) * 128, :])
                cwT.append(t)
            c0S = []
            ms0S = []
            for nm, dram, lst in (("c0Sa", c0Sa_d, c0S), ("c0Sb", c0Sb_d, c0S),
                                  ("m0Sa", ms0Sa_d, ms0S), ("m0Sb", ms0Sb_d, ms0S)):
                t = cpool.tile([128, STATE], DT, tag=nm)
                nc.sync.dma_start(t[:], dram[:, :])
                lst.append(t)
            c0T3 = cpool.tile([128, NT, GL], DT, tag="c0T3")
            nc.sync.dma_start(c0T3[:], c0T3_d[:, :, :])
            s0T3 = cpool.tile([128, NT, GL], DT, tag="s0T3")
            nc.sync.dma_start(s0T3[:], s0T3_d[:, :, :])
            cL4 = cpool.tile([128, NT], DT, tag="cL4")
            nc.sync.dma_start(cL4[:], cL4_d[:, :])
            sL4 = cpool.tile([128, NT], DT, tag="sL4")
            nc.sync.dma_start(sL4[:], sL4_d[:, :])
            UO = cpool.tile([128, 256], F32R, tag="UO")
            nc.sync.dma_start(UO[:], UO_d[:, :])
            ZU = cpool.tile([128, 256], F32R, tag="ZU")
            nc.sync.dma_start(ZU[:], ZU_d[:, :])

            qre = cpool.tile([128, NT], DT, tag="q0re")
            qim = cpool.tile([128, NT], DT, tag="q0im")
            nc.vector.memset(qre[:], 0.0)
            nc.vector.memset(qim[:], 0.0)

            add = mybir.AluOpType.add
            sub = mybir.AluOpType.subtract
            mult = mybir.AluOpType.mult
            CP = mybir.ActivationFunctionType.Identity

            for g in range(NGROUP):
                # ---- load uT group (8 d-tiles x (128, 512)) ----
                ut = []
                for k in range(8):
                    t = upool.tile([128, GL], F32R, tag=f"ut{k}")
                    nc.sync.dma_start(
                        t[:], uT_d[k * 128:(k + 1) * 128, g * GL:(g + 1) * GL])
                    ut.append(t)

                # ---- per sigma-subtile: Bu matmul + rotation ----
                vre = []
                vim = []
                for s4 in range(4):
                    bu_ps = psbu.tile([128, STATE], DT, tag="bu")
                    for k in range(8):
                        nc.tensor.matmul(
                            bu_ps[:],
                            ut[k][:, s4 * 128:(s4 + 1) * 128],
                            bwT[k][:],
                            start=(k == 0), stop=(k == 7))
                    buS = bupool.tile([128, STATE], DT, tag="buS")
                    nc.scalar.activation(buS[:], bu_ps[:], CP)
                    vr = vpool.tile([128, STATE], F32R, tag="vre")
                    nc.vector.tensor_mul(vr[:], buS[:], c0S[s4 % 2][:])
                    vi = vpool.tile([128, STATE], F32R, tag="vim")
                    nc.gpsimd.tensor_mul(vi[:], buS[:], ms0S[s4 % 2][:])
                    vre.append(vr)
                    vim.append(vi)

                are3 = apool.tile([128, NT, GL], DT, tag="are")
                aim3 = apool.tile([128, NT, GL], DT, tag="aim")

                for c in range(2):  # two 256-chunks in the group
                    va, vb = (vre[2 * c], vre[2 * c + 1])
                    wa, wb = (vim[2 * c], vim[2 * c + 1])
                    for comp, (x0, x1, a3, q) in enumerate(
                            (((va, vb, are3, qre)), ((wa, wb, aim3, qim)))):
                        for pr in range(2):  # ntile pairs
                            w_ps = psw.tile([128, GL], DT, tag=f"w{comp}")
                            for half in range(2):
                                ntile = 2 * pr + half
                                colr = slice(half * 256, half * 256 + 256)
                                nc.tensor.matmul(
                                    w_ps[:, colr],
                                    x0[:, ntile * 128:(ntile + 1) * 128],
                                    UO[:], start=True, stop=False)
                                nc.tensor.matmul(
                                    w_ps[:, colr],
                                    x1[:, ntile * 128:(ntile + 1) * 128],
                                    ZU[:], start=False, stop=True)
                                # bias-move: A = W + Q  (per-partition bias)
                                nc.scalar.activation(
                                    a3[:, ntile, c * 256:(c + 1) * 256],
                                    w_ps[:, colr], CP,
                                    bias=q[:, ntile:ntile + 1])
                    # ---- carry update: Q' = lam^L (Q + R), T = A[..., last] ----
                    off = c * 256 + 255
                    tre = are3[:, :, off]
                    tim = aim3[:, :, off]
                    u1re = qpool.tile([128, NT], DT, tag="u1re")
                    nc.vector.tensor_mul(u1re[:], tre, cL4[:])
                    u1im = qpool.tile([128, NT], DT, tag="u1im")
                    nc.vector.tensor_mul(u1im[:], tim, cL4[:])
                    vv = qpool.tile([128, NT], DT, tag="vv")
                    nc.vector.tensor_mul(vv[:], tim, sL4[:])
                    w2 = qpool.tile([128, NT], DT, tag="w2")
                    nc.vector.tensor_mul(w2[:], tre, sL4[:])
                    qre = qpool.tile([128, NT], DT, tag="qre")
                    nc.vector.tensor_sub(qre[:], u1re[:], vv[:])
                    qim = qpool.tile([128, NT], DT, tag="qim")
                    nc.vector.tensor_add(qim[:], u1im[:], w2[:])

                # ---- combine + projection, split per 256-chunk ----
                p1 = ppool.tile([128, NT, GL], DT, tag="p1")
                p2 = ppool.tile([128, NT, GL], DT, tag="p2")
                hT3 = hpool.tile([128, NT, GL], F32R, tag="h")
                for cc in range(2):
                    cr = slice(cc * 256, (cc + 1) * 256)
                    nc.vector.tensor_mul(p1[:, :, cr], are3[:, :, cr], c0T3[:, :, cr])
                    nc.gpsimd.tensor_mul(p2[:, :, cr], aim3[:, :, cr], s0T3[:, :, cr])
                    nc.vector.tensor_sub(hT3[:, :, cr], p1[:, :, cr], p2[:, :, cr])
                yps = []
                for mt in range(4):
                    y_ps = psy.tile([128, GL], DT, tag="y")
                    yps.append(y_ps)
                yo_t = []
                for mt in range(4):
                    yot = ypool.tile([128, GL], DT, tag=f"yo{mt}")
                    yo_t.append(yot)
                for cc in range(2):
                    cr = slice(cc * 256, (cc + 1) * 256)
                    for mt in range(4):
                        for kt in range(4):
                            nc.tensor.matmul(
                                yps[mt][:, cr],
                                cwT[kt][:, mt * 128:(mt + 1) * 128],
                                hT3[:, kt, cr],
                                start=(kt == 0), stop=(kt == 3))
                        nc.vector.tensor_add(
                            yo_t[mt][:, cr], yps[mt][:, cr],
                            ut[mt][:, cr].bitcast(DT))
                        nc.sync.dma_start(
                            yT_d[mt * 128:(mt + 1) * 128,
                                 g * GL + cc * 256:g * GL + (cc + 1) * 256],
                            yo_t[mt][:, cr])
    nc.compile()
    return nc


def _host_tables(a_params):
    n = STATE
    half = n // 2
    a_full = np.zeros(n)
    a_full[1:half + 1] = a_params.astype(np.float64)
    a_full[half + 1:] = -a_params.astype(np.float64)[::-1][: n - half - 1]
    omega = np.imag(np.fft.fft(a_full))
    theta = -2.0 * np.arctan(omega)          # (512,)
    sig = np.arange(256)
    cS = np.cos(sig[:, None] * theta[None, :])
    sS = np.sin(sig[:, None] * theta[None, :])
    tabs = {
        "c0Sa": cS[:128], "c0Sb": cS[128:],
        "ms0Sa": -sS[:128], "ms0Sb": -sS[128:],
    }
    # (128, NT, GL) combine tables: [p, nt, tg] = trig((tg % 256) * theta[128*nt+p])
    tg = np.arange(GL) % 256
    c0T3 = np.empty((128, NT, GL))
    s0T3 = np.empty((128, NT, GL))
    for nt in range(NT):
        th = theta[128 * nt:128 * (nt + 1)]
        c0T3[:, nt, :] = np.cos(th[:, None] * tg[None, :])
        s0T3[:, nt, :] = np.sin(th[:, None] * tg[None, :])
    tabs["c0T3"] = c0T3
    tabs["s0T3"] = s0T3
    thL = theta.reshape(NT, 128).T * L       # (128, NT)
    tabs["cL4"] = np.cos(thL)
    tabs["sL4"] = np.sin(thL)
    U = np.triu(np.ones((128, 128)))
    tabs["UO"] = np.concatenate([U, np.ones((128, 128))], axis=1)
    tabs["ZU"] = np.concatenate([np.zeros((128, 128)), U], axis=1)
    return {k: np.ascontiguousarray(v, dtype=np.float32) for k, v in tabs.items()}


def kernel(u, a_params, B_w, C_w, D, trace=False):
    u = np.asarray(u, dtype=np.float32)
    B_w = np.asarray(B_w, dtype=np.float32)
    C_w = np.asarray(C_w, dtype=np.float32)
    D = np.asarray(D, dtype=np.float32)
    tabs = _host_tables(np.asarray(a_params))

    if "nc" not in _CACHE:
        _CACHE["nc"] = build_nc()
    nc = _CACHE["nc"]

    in_maps = []
    for core in range(8):
        b, hf = core // 2, core % 2
        dperm = np.r_[512:1024, 0:512] if hf else np.r_[0:1024]
        uT = np.ascontiguousarray(u[b].T[dperm])          # (1024, 8192)
        bwT = np.ascontiguousarray(B_w.T[dperm])          # (1024, 512)
        cwT = np.ascontiguousarray(C_w[hf * 512:(hf + 1) * 512].T)  # (512,512)
        # fold D into the skip path is implicit (D==ones); for general D,
        # scale the uT rows used for the skip — D is ones in this problem.
        m = {"uT": uT, "bwT": bwT, "cwT": cwT}
        m.update(tabs)
        in_maps.append(m)

    res = bass_utils.run_bass_kernel_spmd(
        nc, in_maps, core_ids=list(range(8)), trace=trace)
    y = np.empty((BATCH, SEQ, D_MODEL), dtype=np.float32)
    for core in range(8):
        b, hf = core // 2, core % 2
        yT = res.results[core]["yT"]                      # (512, 8192)
        y[b, :, hf * 512:(hf + 1) * 512] = yT.T
    _CACHE["last_res"] = res
    return y



# revision 14
# speedup vs baseline: 1.4252x; 1.4252x over previous
"""Trainium2 Bass kernel for CayleyCirculantSSMLayer (time-split, 2-phase).

Math: lambda_j = (1-i*w_j)/(1+i*w_j) is on the unit circle, so the causal
conv h[t] = sum_{s<=t} Re(lambda^{t-s}) Bu[s] factors through a rotated
cumulative sum chained across 256-row chunks by a per-channel rotation.

Sharding: 8 cores = 4 samples x 2 time-halves. Each core computes Bu +
chunked cumsum for its 4096-step half with a LOCAL carry chain (phase 1,
storing locally-biased accumulators in SBUF as bf16), the cores of a
pair exchange their final carries with one tiny pair-AllGather, then each
core applies the rotated carry correction, combines with the cos/sin
tables and projects to all 1024 output dims (phase 2). The correction is
identically zero on first-half cores (host-provided mask), keeping one
symmetric SPMD program. All matmuls/elementwise run in bf16 (f32 psum).
"""
import sys
import numpy as np
import ml_dtypes

for p in ("/opt/trn_rl_repo",):
    if p not in sys.path:
        sys.path.insert(0, p)

from concourse import bass, bacc, mybir, tile
from concourse import bass_utils

D_MODEL = 1024
STATE = 512
BATCH = 4
SEQ = 8192
TH = SEQ // 2             # per-core time half
L = 256                   # carry-chunk length
GL = 512                  # group length = 2 chunks
NG = TH // GL             # 8 groups per core
NCH = TH // L             # 16 chunks per core
NT = 4                    # state n-tiles of 128
DT = mybir.dt.float32
BF = mybir.dt.bfloat16
BF_NP = ml_dtypes.bfloat16

_CACHE = {}


def build_nc():
    nc = bacc.Bacc(None, target_bir_lowering=False, num_devices=8)
    uT_d = nc.dram_tensor("uT", [128, 8, TH], BF, kind="ExternalInput")
    bwT_d = nc.dram_tensor("bwT", [128, 8, STATE], BF, kind="ExternalInput")
    cwT_d = nc.dram_tensor("cwT", [128, 4, D_MODEL], BF, kind="ExternalInput")
    dD_d = nc.dram_tensor("dD", [128, D_MODEL], BF, kind="ExternalInput")
    csP_d = nc.dram_tensor("csP", [128, 4, STATE], BF, kind="ExternalInput")
    c0T3_d = nc.dram_tensor("c0T3", [128, NT, GL], BF, kind="ExternalInput")
    s0T3_d = nc.dram_tensor("s0T3", [128, NT, GL], BF, kind="ExternalInput")
    cL4_d = nc.dram_tensor("cL4", [128, NT], DT, kind="ExternalInput")
    sL4_d = nc.dram_tensor("sL4", [128, NT], DT, kind="ExternalInput")
    rotC_d = nc.dram_tensor("rotC", [128, NCH * NT], DT, kind="ExternalInput")
    rotS_d = nc.dram_tensor("rotS", [128, NCH * NT], DT, kind="ExternalInput")
    U1_d = nc.dram_tensor("U1", [128, 256], BF, kind="ExternalInput")
    # output layout: [row-in-mt (=partition), mt, time]
    yT_d = nc.dram_tensor("yT", [128, 8, TH], BF, kind="ExternalOutput")

    cc_in_d = nc.dram_tensor("cc_in", [128, 8], DT)
    cc_out_d = nc.dram_tensor("cc_out", [2, 128, 8], DT)

    add = mybir.AluOpType.add
    mult = mybir.AluOpType.mult
    CP = mybir.ActivationFunctionType.Identity

    with tile.TileContext(nc) as tc:
        with (
            tc.tile_pool(name="const", bufs=1) as cpool,
            tc.tile_pool(name="ut", bufs=3) as upool,
            tc.tile_pool(name="bus", bufs=4) as bupool,
            tc.tile_pool(name="v", bufs=4) as vpool,
            tc.tile_pool(name="qc", bufs=2) as qpool,
            tc.tile_pool(name="pgrp", bufs=2) as ppool,
            tc.tile_pool(name="hgrp", bufs=2) as hpool,
            tc.tile_pool(name="yo", bufs=4) as ypool,
        ):
            # ---- phase-1-critical constants first (unblock first Bu fast) --
            bwT3 = cpool.tile([128, 8, STATE], BF, tag="bwT3")
            nc.sync.dma_start(bwT3[:], bwT_d[:, :, :])

            csPack = cpool.tile([128, 4, STATE], BF, tag="csPack")
            nc.sync.dma_start(csPack[:], csP_d[:, :, :])

            U1 = cpool.tile([128, 256], BF, tag="U1")
            nc.sync.dma_start(U1[:], U1_d[:, :])
            cL4 = cpool.tile([128, NT], DT, tag="cL4")
            nc.sync.dma_start(cL4[:], cL4_d[:, :])
            sL4 = cpool.tile([128, NT], DT, tag="sL4")
            nc.sync.dma_start(sL4[:], sL4_d[:, :])

            # first u double-group before the remaining constants
            uts = {}

            def load_u(g, pool_tag, split=False):
                t = upool.tile([128, 8, GL], BF, tag=pool_tag,
                               name=f"{pool_tag}_{g}")
                if split:
                    nc.sync.dma_start(t[:, :, 0:GL // 2],
                                      uT_d[:, :, g * GL:g * GL + GL // 2])
                    nc.sync.dma_start(t[:, :, GL // 2:GL],
                                      uT_d[:, :, g * GL + GL // 2:(g + 1) * GL])
                else:
                    nc.sync.dma_start(
                        t[:], uT_d[:, :, g * GL:(g + 1) * GL])
                return t

            uts[0] = load_u(0, "ut", split=True)

            # ---- remaining constants (needed later than first Bu) ----
            c0T3 = cpool.tile([128, NT, GL], BF, tag="c0T3")
            nc.sync.dma_start(c0T3[:], c0T3_d[:, :, :])
            s0T3 = cpool.tile([128, NT, GL], BF, tag="s0T3")
            nc.sync.dma_start(s0T3[:], s0T3_d[:, :, :])
            cwT3 = cpool.tile([128, 4, D_MODEL], BF, tag="cwT3")
            nc.sync.dma_start(cwT3[:], cwT_d[:, :, :])

            dD = cpool.tile([128, D_MODEL], BF, tag="dD")
            nc.sync.dma_start(dD[:], dD_d[:, :])
            rotC = cpool.tile([128, NCH * NT], DT, tag="rotC")
            nc.sync.dma_start(rotC[:], rotC_d[:, :])
            rotS = cpool.tile([128, NCH * NT], DT, tag="rotS")
            nc.sync.dma_start(rotS[:], rotS_d[:, :])

            # persistent per-group accumulators (locally-biased), bf16
            W3 = [[cpool.tile([128, NT, GL], BF, tag=f"w3_{g}_{comp}",
                              name=f"w3_{g}_{comp}")
                   for comp in range(2)] for g in range(NG)]

            qre = cpool.tile([128, NT], DT, tag="q0re")
            qim = cpool.tile([128, NT], DT, tag="q0im")
            nc.vector.memset(qre[:], 0.0)
            nc.vector.memset(qim[:], 0.0)

            # =================== PHASE 1 ===================
            def emit_bu(ut, goff, s4):
                bu_ps = psbu.tile([128, STATE], DT, tag="bu")
                co = goff * GL + s4 * 128
                for k in range(8):
                    nc.tensor.matmul(
                        bu_ps[:], ut[:, k, co:co + 128], bwT3[:, k, :],
                        start=(k == 0), stop=(k == 7))
                buS = bupool.tile([128, STATE], BF, tag="buS")
                nc.vector.tensor_copy(out=buS[:], in_=bu_ps[:])
                vr = vpool.tile([128, STATE], BF, tag="vre")
                nc.vector.tensor_mul(vr[:], buS[:], csPack[:, s4 % 2, :])
                vi = vpool.tile([128, STATE], BF, tag="vim")
                nc.gpsimd.tensor_mul(vi[:], buS[:], csPack[:, 2 + s4 % 2, :])
                return vr, vi

            def emit_chunk(g, c, vre, vim):
                nonlocal qre, qim
                wlast = []
                for comp, xs in ((0, vre), (1, vim)):
                    wl = qpool.tile([128, NT], DT, tag=f"wl{comp}")
                    for pr in range(2):
                        w_ps = psw.tile([128, GL], DT, tag=f"w{comp}")
                        for half in range(2):
                            ntl = 2 * pr + half
                            b = half * 256
                            x0 = xs[2 * c][:, ntl * 128:(ntl + 1) * 128]
                            x1 = xs[2 * c + 1][:, ntl * 128:(ntl + 1) * 128]
                            nc.tensor.matmul(w_ps[:, b:b + 128], x0,
                                             U1[:, 0:128], start=True, stop=True)
                            nc.tensor.matmul(w_ps[:, b + 128:b + 256], x0,
                                             U1[:, 128:256], start=True, stop=False)
                            nc.tensor.matmul(w_ps[:, b + 128:b + 256], x1,
                                             U1[:, 0:128], start=False, stop=True)
                        # chunk-local sums (pre-bias) for the carry chain
                        nc.vector.tensor_copy(
                            out=wl[:, 2 * pr:2 * pr + 2],
                            in_=w_ps[:, 255:512:256])
                        # biased move PSUM -> SBUF (bf16)
                        for half in range(2):
                            ntl = 2 * pr + half
                            q = qre if comp == 0 else qim
                            nc.scalar.activation(
                                W3[g][comp][:, ntl, c * 256:(c + 1) * 256],
                                w_ps[:, half * 256:(half + 1) * 256], CP,
                                bias=q[:, ntl:ntl + 1])
                    wlast.append(wl)
                # carry chain: q' = lam^L (Wlast + q)
                tre = qpool.tile([128, NT], DT, tag="tre")
                nc.vector.tensor_add(tre[:], wlast[0][:], qre[:])
                tim = qpool.tile([128, NT], DT, tag="tim")
                nc.vector.tensor_add(tim[:], wlast[1][:], qim[:])
                u1re = qpool.tile([128, NT], DT, tag="u1re")
                nc.vector.tensor_mul(u1re[:], tre[:], cL4[:])
                vv = qpool.tile([128, NT], DT, tag="vv")
                nc.vector.tensor_mul(vv[:], tim[:], sL4[:])
                u1im = qpool.tile([128, NT], DT, tag="u1im")
                nc.vector.tensor_mul(u1im[:], tim[:], cL4[:])
                w2 = qpool.tile([128, NT], DT, tag="w2")
                nc.vector.tensor_mul(w2[:], tre[:], sL4[:])
                qre = qpool.tile([128, NT], DT, tag="qre")
                nc.vector.tensor_sub(qre[:], u1re[:], vv[:])
                qim = qpool.tile([128, NT], DT, tag="qim")
                nc.vector.tensor_add(qim[:], u1im[:], w2[:])

            with (
                tc.tile_pool(name="ps_bu", bufs=2, space="PSUM") as psbu,
                tc.tile_pool(name="ps_w", bufs=3, space="PSUM") as psw,
            ):
                for g in range(NG):
                    if g not in uts:
                        uts[g] = load_u(g, "ut")
                    if g + 1 < NG and g + 1 not in uts:
                        uts[g + 1] = load_u(g + 1, "ut")
                    ut = uts[g]
                    goff = 0
                    vre, vim = [], []
                    for s4 in range(4):
                        vr, vi = emit_bu(ut, goff, s4)
                        vre.append(vr)
                        vim.append(vi)
                    emit_chunk(g, 0, vre, vim)
                    emit_chunk(g, 1, vre, vim)

            # =================== CARRY EXCHANGE ===================
            qpack = cpool.tile([128, 8], DT, tag="qpack")
            nc.vector.tensor_copy(out=qpack[:, 0:4], in_=qre[:])
            nc.vector.tensor_copy(out=qpack[:, 4:8], in_=qim[:])
            nc.sync.dma_start(cc_in_d[:, :], qpack[:])

            # prefetch first phase-2 u double-group (independent of collective)
            ut2s = {0: load_u(0, "ut"), 1: load_u(1, "ut")}

            nc.gpsimd.collective_compute(
                "AllGather", mybir.AluOpType.bypass,
                replica_groups=[[0, 1], [2, 3], [4, 5], [6, 7]],
                ins=[cc_in_d[:, :].opt()], outs=[cc_out_d[:, :, :].opt()])
            qhand = cpool.tile([128, 8], DT, tag="qhand")
            nc.sync.dma_start(qhand[:], cc_out_d[0, :, :])

            # broadcast (128,4) -> (128,64) by log-doubling, then rotate
            qrep = []
            for comp in range(2):
                t = cpool.tile([128, NCH * NT], DT, tag=f"qrep{comp}",
                               name=f"qrep{comp}")
                nc.vector.tensor_copy(out=t[:, 0:4],
                                      in_=qhand[:, 4 * comp:4 * comp + 4])
                for w in (4, 8, 16, 32):
                    nc.vector.tensor_copy(out=t[:, w:2 * w], in_=t[:, 0:w])
                qrep.append(t)
            t1 = cpool.tile([128, NCH * NT], DT, tag="d_t1")
            nc.vector.tensor_mul(t1[:], rotC[:], qrep[0][:])
            t2 = cpool.tile([128, NCH * NT], DT, tag="d_t2")
            nc.vector.tensor_mul(t2[:], rotS[:], qrep[1][:])
            Dre = cpool.tile([128, NCH * NT], DT, tag="Dre")
            nc.vector.tensor_sub(Dre[:], t1[:], t2[:])
            t3 = cpool.tile([128, NCH * NT], DT, tag="d_t3")
            nc.vector.tensor_mul(t3[:], rotC[:], qrep[1][:])
            t4 = cpool.tile([128, NCH * NT], DT, tag="d_t4")
            nc.vector.tensor_mul(t4[:], rotS[:], qrep[0][:])
            Dim = cpool.tile([128, NCH * NT], DT, tag="Dim")
            nc.vector.tensor_add(Dim[:], t3[:], t4[:])

            # =================== PHASE 2 ===================
            with tc.tile_pool(name="ps_y", bufs=1, space="PSUM") as psy:
                for g in range(NG):
                    if g not in ut2s:
                        ut2s[g] = load_u(g, "ut")
                    if g + 1 < NG and g + 1 not in ut2s:
                        ut2s[g + 1] = load_u(g + 1, "ut")
                    ut2 = ut2s[g]
                    goff = 0
                    p1 = ppool.tile([128, NT, GL], BF, tag="p1")
                    p2 = ppool.tile([128, NT, GL], BF, tag="p2")
                    hT3 = hpool.tile([128, NT, GL], BF, tag="h")
                    for c2 in range(2):
                        cr = slice(c2 * 256, (c2 + 1) * 256)
                        for ntl in range(NT):
                            idx = (2 * g + c2) * NT + ntl
                            nc.vector.scalar_tensor_tensor(
                                p1[:, ntl, cr], W3[g][0][:, ntl, cr],
                                Dre[:, idx:idx + 1], c0T3[:, ntl, cr], add, mult)
                            nc.vector.scalar_tensor_tensor(
                                p2[:, ntl, cr], W3[g][1][:, ntl, cr],
                                Dim[:, idx:idx + 1], s0T3[:, ntl, cr], add, mult)
                    nc.gpsimd.tensor_sub(hT3[:], p1[:], p2[:])
                    # projection: y_ps (128, 1024) = [mt_local(2) x cc(2) x 256]
                    for mp in range(4):
                        y_ps = psy.tile([128, 2 * GL], DT, tag=f"y{mp}",
                                        name=f"y{mp}_{g}")
                        for half in range(2):
                            mt = 2 * mp + half
                            for cc in range(2):
                                b = half * 512 + cc * 256
                                cr = slice(cc * 256, (cc + 1) * 256)
                                for kt in range(4):
                                    nc.tensor.matmul(
                                        y_ps[:, b:b + 256],
                                        cwT3[:, kt, mt * 128:(mt + 1) * 128],
                                        hT3[:, kt, cr],
                                        start=(kt == 0), stop=False)
                                nc.tensor.matmul(
                                    y_ps[:, b:b + 256],
                                    dD[:, mt * 128:(mt + 1) * 128],
                                    ut2[:, mt, cc * 256:(cc + 1) * 256],
                                    start=False, stop=True)
                        yo = ypool.tile([128, 2 * GL], BF, tag="yo",
                                        name=f"yo{mp}_{g}")
                        nc.scalar.copy(yo[:], y_ps[:])
                        nc.sync.dma_start(
                            yT_d[:, 2 * mp:2 * mp + 2, g * GL:(g + 1) * GL],
                            yo[:])
    nc.compile()
    return nc


def _host_tables(a_params):
    n = STATE
    half = n // 2
    a_full = np.zeros(n)
    a_full[1:half + 1] = a_params.astype(np.float64)
    a_full[half + 1:] = -a_params.astype(np.float64)[::-1][: n - half - 1]
    omega = np.imag(np.fft.fft(a_full))
    theta = -2.0 * np.arctan(omega)          # (512,)
    sig = np.arange(256)
    cS = np.cos(sig[:, None] * theta[None, :])
    sS = np.sin(sig[:, None] * theta[None, :])
    tabs = {
        "c0Sa": cS[:128], "c0Sb": cS[128:],
        "ms0Sa": -sS[:128], "ms0Sb": -sS[128:],
    }
    tg = np.arange(GL) % 256
    c0T3 = np.empty((128, NT, GL))
    s0T3 = np.empty((128, NT, GL))
    for nt in range(NT):
        th = theta[128 * nt:128 * (nt + 1)]
        c0T3[:, nt, :] = np.cos(th[:, None] * tg[None, :])
        s0T3[:, nt, :] = np.sin(th[:, None] * tg[None, :])
    tabs["c0T3"] = c0T3
    tabs["s0T3"] = s0T3
    thL = theta.reshape(NT, 128).T * L       # (128, NT)
    tabs["cL4"] = np.cos(thL)
    tabs["sL4"] = np.sin(thL)
    # rot tables for the cross-core carry correction: lam^(L*c), c=0..NCH-1
    thP = theta.reshape(NT, 128).T           # (128, NT)
    rotC = np.empty((128, NCH * NT))
    rotS = np.empty((128, NCH * NT))
    for c in range(NCH):
        rotC[:, c * NT:(c + 1) * NT] = np.cos(thP * (L * c))
        rotS[:, c * NT:(c + 1) * NT] = np.sin(thP * (L * c))
    tabs["rotC"] = rotC
    tabs["rotS"] = rotS
    U = np.triu(np.ones((128, 128)))
    tabs["U1"] = np.concatenate([U, np.ones((128, 128))], axis=1)
    return tabs


def kernel(u, a_params, B_w, C_w, D, trace=False):
    u = np.asarray(u, dtype=np.float32)
    B_w = np.asarray(B_w, dtype=np.float32)
    C_w = np.asarray(C_w, dtype=np.float32)
    D = np.asarray(D, dtype=np.float32)
    tabs = _host_tables(np.asarray(a_params))

    if "nc" not in _CACHE:
        _CACHE["nc"] = build_nc()
    nc = _CACHE["nc"]

    bf_tabs = {}
    for k, v in tabs.items():
        dt = np.float32 if k in ("cL4", "sL4", "rotC", "rotS") else BF_NP
        bf_tabs[k] = np.ascontiguousarray(v.astype(dt))
    # pack sigma-rotation tables: csP (128, 4, 512)
    csP = np.stack([bf_tabs.pop("c0Sa"), bf_tabs.pop("c0Sb"),
                    bf_tabs.pop("ms0Sa"), bf_tabs.pop("ms0Sb")], axis=1)
    bf_tabs["csP"] = np.ascontiguousarray(csP)

    # bwT: (128 p, 8 k, 512 ch) with d = k*128 + p
    bwT = np.ascontiguousarray(
        B_w.T.reshape(8, 128, STATE).transpose(1, 0, 2).astype(BF_NP))
    # cwT: (128 p, 4 kt, 1024 d) with ch = kt*128 + p
    cwT = np.ascontiguousarray(
        C_w.T.reshape(4, 128, D_MODEL).transpose(1, 0, 2).astype(BF_NP))
    dD = np.zeros((128, D_MODEL), dtype=BF_NP)
    for mt in range(8):
        blk = np.diag(D[mt * 128:(mt + 1) * 128])
        dD[:, mt * 128:(mt + 1) * 128] = blk.astype(BF_NP)

    in_maps = []
    for core in range(8):
        b, hf = core // 2, core % 2
        # uT: (128 p, 8 k, TH) with d = k*128 + p
        uT = np.ascontiguousarray(
            u[b, hf * TH:(hf + 1) * TH, :].T.reshape(8, 128, TH)
            .transpose(1, 0, 2).astype(BF_NP))
        # fold the half-mask into the correction rot tables
        rotC = np.ascontiguousarray(bf_tabs["rotC"] * float(hf))
        rotS = np.ascontiguousarray(bf_tabs["rotS"] * float(hf))
        m = {"uT": uT, "bwT": bwT, "cwT": cwT, "dD": dD,
             "rotC": rotC, "rotS": rotS}
        for k2, v2 in bf_tabs.items():
            if k2 not in ("rotC", "rotS"):
                m[k2] = v2
        in_maps.append(m)

    res = bass_utils.run_bass_kernel_spmd(
        nc, in_maps, core_ids=list(range(8)), trace=trace)
    y = np.empty((BATCH, SEQ, D_MODEL), dtype=np.float32)
    for core in range(8):
        b, hf = core // 2, core % 2
        yT = np.asarray(res.results[core]["yT"]).astype(np.float32)  # (128,8,TH)
        # y[b, t, mt*128 + p] = yT[p, mt, t]
        y[b, hf * TH:(hf + 1) * TH, :] = yT.transpose(2, 1, 0).reshape(TH, D_MODEL)
    _CACHE["last_res"] = res
    return y


# revision 17
# speedup vs baseline: 1.5048x; 1.0559x over previous
"""Trainium2 Bass kernel for CayleyCirculantSSMLayer (time-split, 2-phase).

Math: lambda_j = (1-i*w_j)/(1+i*w_j) is on the unit circle, so the causal
conv h[t] = sum_{s<=t} Re(lambda^{t-s}) Bu[s] factors through a rotated
cumulative sum chained across 256-row chunks by a per-channel rotation.

Sharding: 8 cores = 4 samples x 2 time-halves. Each core computes Bu +
chunked cumsum for its 4096-step half with a LOCAL carry chain (phase 1,
storing locally-biased accumulators in SBUF as bf16), the cores of a
pair exchange their final carries with one tiny pair-AllGather, then each
core applies the rotated carry correction, combines with the cos/sin
tables and projects to all 1024 output dims (phase 2). The correction is
identically zero on first-half cores (host-provided mask), keeping one
symmetric SPMD program. All matmuls/elementwise run in bf16 (f32 psum).
"""
import sys
import numpy as np
import ml_dtypes

for p in ("/opt/trn_rl_repo",):
    if p not in sys.path:
        sys.path.insert(0, p)

from concourse import bass, bacc, mybir, tile
from concourse import bass_utils

D_MODEL = 1024
STATE = 512
BATCH = 4
SEQ = 8192
TH = SEQ // 2             # per-core time half
L = 256                   # carry-chunk length
GL = 512                  # group length = 2 chunks
NG = TH // GL             # 8 groups per core
NCH = TH // L             # 16 chunks per core
NT = 4                    # state n-tiles of 128
DT = mybir.dt.float32
BF = mybir.dt.bfloat16
BF_NP = ml_dtypes.bfloat16

_CACHE = {}


def build_nc():
    nc = bacc.Bacc(None, target_bir_lowering=False, num_devices=8)
    uT_d = nc.dram_tensor("uT", [128, 8, TH], BF, kind="ExternalInput")
    bwT_d = nc.dram_tensor("bwT", [128, 8, STATE], BF, kind="ExternalInput")
    cwT_d = nc.dram_tensor("cwT", [128, 4, D_MODEL], BF, kind="ExternalInput")
    dD_d = nc.dram_tensor("dD", [128, D_MODEL], BF, kind="ExternalInput")
    csP_d = nc.dram_tensor("csP", [128, 4, STATE], BF, kind="ExternalInput")
    c0T3_d = nc.dram_tensor("c0T3", [128, NT, GL], BF, kind="ExternalInput")
    s0T3_d = nc.dram_tensor("s0T3", [128, NT, GL], BF, kind="ExternalInput")
    cL4_d = nc.dram_tensor("cL4", [128, NT], DT, kind="ExternalInput")
    sL4_d = nc.dram_tensor("sL4", [128, NT], DT, kind="ExternalInput")
    rotC_d = nc.dram_tensor("rotC", [128, NCH * NT], DT, kind="ExternalInput")
    rotS_d = nc.dram_tensor("rotS", [128, NCH * NT], DT, kind="ExternalInput")
    U1_d = nc.dram_tensor("U1", [128, 256], BF, kind="ExternalInput")
    # output layout: [row-in-mt (=partition), mt, time]
    yT_d = nc.dram_tensor("yT", [128, 8, TH], BF, kind="ExternalOutput")

    cc_in_d = nc.dram_tensor("cc_in", [128, 8], DT)
    cc_out_d = nc.dram_tensor("cc_out", [2, 128, 8], DT)

    add = mybir.AluOpType.add
    mult = mybir.AluOpType.mult
    CP = mybir.ActivationFunctionType.Identity

    with tile.TileContext(nc) as tc:
        with (
            tc.tile_pool(name="const", bufs=1) as cpool,
            tc.tile_pool(name="ut", bufs=3) as upool,
            tc.tile_pool(name="bus", bufs=4) as bupool,
            tc.tile_pool(name="v", bufs=4) as vpool,
            tc.tile_pool(name="qc", bufs=2) as qpool,
            tc.tile_pool(name="pgrp", bufs=2) as ppool,
            tc.tile_pool(name="hgrp", bufs=2) as hpool,
            tc.tile_pool(name="yo", bufs=4) as ypool,
        ):
            # ---- phase-1-critical constants first (unblock first Bu fast) --
            bwT3 = cpool.tile([128, 8, STATE], BF, tag="bwT3")
            nc.sync.dma_start(bwT3[:, 0:4, :], bwT_d[:, 0:4, :])
            nc.sync.dma_start(bwT3[:, 4:8, :], bwT_d[:, 4:8, :])

            csPack = cpool.tile([128, 4, STATE], BF, tag="csPack")
            nc.sync.dma_start(csPack[:], csP_d[:, :, :])

            U1 = cpool.tile([128, 256], BF, tag="U1")
            nc.sync.dma_start(U1[:], U1_d[:, :])
            cL4 = cpool.tile([128, NT], DT, tag="cL4")
            nc.sync.dma_start(cL4[:], cL4_d[:, :])
            sL4 = cpool.tile([128, NT], DT, tag="sL4")
            nc.sync.dma_start(sL4[:], sL4_d[:, :])

            # first u double-group before the remaining constants
            uts = {}

            def load_u(g, pool_tag, split=False):
                t = upool.tile([128, 8, GL], BF, tag=pool_tag,
                               name=f"{pool_tag}_{g}")
                if split:
                    nc.sync.dma_start(t[:, :, 0:GL // 2],
                                      uT_d[:, :, g * GL:g * GL + GL // 2])
                    nc.sync.dma_start(t[:, :, GL // 2:GL],
                                      uT_d[:, :, g * GL + GL // 2:(g + 1) * GL])
                else:
                    nc.sync.dma_start(
                        t[:], uT_d[:, :, g * GL:(g + 1) * GL])
                return t

            uts[0] = load_u(0, "ut", split=True)
            nc.sync.dma_start(bwT3[:, 4:8, :], bwT_d[:, 4:8, :])
            csPack = cpool.tile([128, 4, STATE], BF, tag="csPack")
            nc.sync.dma_start(csPack[:], csP_d[:, :, :])
            U1 = cpool.tile([128, 256], BF, tag="U1")
            nc.sync.dma_start(U1[:], U1_d[:, :])
            cL4 = cpool.tile([128, NT], DT, tag="cL4")
            nc.sync.dma_start(cL4[:], cL4_d[:, :])
            sL4 = cpool.tile([128, NT], DT, tag="sL4")
            nc.sync.dma_start(sL4[:], sL4_d[:, :])

            # ---- remaining constants (needed later than first Bu) ----
            c0T3 = cpool.tile([128, NT, GL], BF, tag="c0T3")
            nc.sync.dma_start(c0T3[:], c0T3_d[:, :, :])
            s0T3 = cpool.tile([128, NT, GL], BF, tag="s0T3")
            nc.sync.dma_start(s0T3[:], s0T3_d[:, :, :])
            cwT3 = cpool.tile([128, 4, D_MODEL], BF, tag="cwT3")
            nc.sync.dma_start(cwT3[:], cwT_d[:, :, :])

            dD = cpool.tile([128, D_MODEL], BF, tag="dD")
            nc.sync.dma_start(dD[:], dD_d[:, :])
            rotC = cpool.tile([128, NCH * NT], DT, tag="rotC")
            nc.sync.dma_start(rotC[:], rotC_d[:, :])
            rotS = cpool.tile([128, NCH * NT], DT, tag="rotS")
            nc.sync.dma_start(rotS[:], rotS_d[:, :])

            # persistent per-group accumulators (locally-biased), bf16
            W3 = [[cpool.tile([128, NT, GL], BF, tag=f"w3_{g}_{comp}",
                              name=f"w3_{g}_{comp}")
                   for comp in range(2)] for g in range(NG)]

            qre = cpool.tile([128, NT], DT, tag="q0re")
            qim = cpool.tile([128, NT], DT, tag="q0im")
            nc.vector.memset(qre[:], 0.0)
            nc.vector.memset(qim[:], 0.0)

            # =================== PHASE 1 ===================
            def emit_bu(ut, goff, s4):
                bu_ps = psbu.tile([128, STATE], DT, tag="bu")
                co = goff * GL + s4 * 128
                for k in range(8):
                    nc.tensor.matmul(
                        bu_ps[:], ut[:, k, co:co + 128], bwT3[:, k, :],
                        start=(k == 0), stop=(k == 7))
                buS = bupool.tile([128, STATE], BF, tag="buS")
                nc.vector.tensor_copy(out=buS[:], in_=bu_ps[:])
                vr = vpool.tile([128, STATE], BF, tag="vre")
                nc.vector.tensor_mul(vr[:], buS[:], csPack[:, s4 % 2, :])
                vi = vpool.tile([128, STATE], BF, tag="vim")
                nc.gpsimd.tensor_mul(vi[:, 0:256], buS[:, 0:256],
                                     csPack[:, 2 + s4 % 2, 0:256])
                nc.gpsimd.tensor_mul(vi[:, 256:512], buS[:, 256:512],
                                     csPack[:, 2 + s4 % 2, 256:512])
                return vr, vi

            def emit_chunk(g, c, vre, vim):
                nonlocal qre, qim
                wlast = []
                for comp, xs in ((0, vre), (1, vim)):
                    wl = qpool.tile([128, NT], DT, tag=f"wl{comp}")
                    for pr in range(2):
                        w_ps = psw.tile([128, GL], DT, tag=f"w{comp}")
                        for half in range(2):
                            ntl = 2 * pr + half
                            b = half * 256
                            x0 = xs[2 * c][:, ntl * 128:(ntl + 1) * 128]
                            x1 = xs[2 * c + 1][:, ntl * 128:(ntl + 1) * 128]
                            nc.tensor.matmul(w_ps[:, b:b + 128], x0,
                                             U1[:, 0:128], start=True, stop=True)
                            nc.tensor.matmul(w_ps[:, b + 128:b + 256], x0,
                                             U1[:, 128:256], start=True, stop=False)
                            nc.tensor.matmul(w_ps[:, b + 128:b + 256], x1,
                                             U1[:, 0:128], start=False, stop=True)
                        # chunk-local sums (pre-bias) for the carry chain
                        nc.vector.tensor_copy(
                            out=wl[:, 2 * pr:2 * pr + 2],
                            in_=w_ps[:, 255:512:256])
                        # biased move PSUM -> SBUF (bf16)
                        for half in range(2):
                            ntl = 2 * pr + half
                            q = qre if comp == 0 else qim
                            nc.scalar.activation(
                                W3[g][comp][:, ntl, c * 256:(c + 1) * 256],
                                w_ps[:, half * 256:(half + 1) * 256], CP,
                                bias=q[:, ntl:ntl + 1])
                    wlast.append(wl)
                # carry chain: q' = lam^L (Wlast + q)
                tre = qpool.tile([128, NT], DT, tag="tre")
                nc.vector.tensor_add(tre[:], wlast[0][:], qre[:])
                tim = qpool.tile([128, NT], DT, tag="tim")
                nc.vector.tensor_add(tim[:], wlast[1][:], qim[:])
                u1re = qpool.tile([128, NT], DT, tag="u1re")
                nc.vector.tensor_mul(u1re[:], tre[:], cL4[:])
                vv = qpool.tile([128, NT], DT, tag="vv")
                nc.vector.tensor_mul(vv[:], tim[:], sL4[:])
                u1im = qpool.tile([128, NT], DT, tag="u1im")
                nc.vector.tensor_mul(u1im[:], tim[:], cL4[:])
                w2 = qpool.tile([128, NT], DT, tag="w2")
                nc.vector.tensor_mul(w2[:], tre[:], sL4[:])
                qre = qpool.tile([128, NT], DT, tag="qre")
                nc.vector.tensor_sub(qre[:], u1re[:], vv[:])
                qim = qpool.tile([128, NT], DT, tag="qim")
                nc.vector.tensor_add(qim[:], u1im[:], w2[:])

            with (
                tc.tile_pool(name="ps_bu", bufs=2, space="PSUM") as psbu,
                tc.tile_pool(name="ps_w", bufs=3, space="PSUM") as psw,
            ):
                for g in range(NG):
                    if g not in uts:
                        uts[g] = load_u(g, "ut")
                    if g + 1 < NG and g + 1 not in uts:
                        uts[g + 1] = load_u(g + 1, "ut")
                    ut = uts[g]
                    goff = 0
                    vre, vim = [], []
                    for s4 in range(4):
                        vr, vi = emit_bu(ut, goff, s4)
                        vre.append(vr)
                        vim.append(vi)
                    emit_chunk(g, 0, vre, vim)
                    emit_chunk(g, 1, vre, vim)

            # =================== CARRY EXCHANGE ===================
            psy_ctx = tc.tile_pool(name="ps_y", bufs=1, space="PSUM")
            psy = psy_ctx.__enter__()
            y_ps_g0 = {}
            qpack = cpool.tile([128, 8], DT, tag="qpack")
            nc.gpsimd.tensor_copy(out=qpack[:, 0:4], in_=qre[:])
            nc.gpsimd.tensor_copy(out=qpack[:, 4:8], in_=qim[:])
            nc.sync.dma_start(cc_in_d[:, :], qpack[:])

            # prefetch first phase-2 u groups (independent of collective)
            ut2s = {0: load_u(0, "ut"), 1: load_u(1, "ut")}
            # pre-run g0's skip matmuls into PSUM during the collective window
            for mp in range(4):
                y_ps = psy.tile([128, 2 * GL], DT, tag=f"y{mp}",
                                name=f"y{mp}_pre")
                y_ps_g0[mp] = y_ps
                for half in range(2):
                    mt = 2 * mp + half
                    for cc in range(2):
                        b = half * 512 + cc * 256
                        nc.tensor.matmul(
                            y_ps[:, b:b + 256],
                            dD[:, mt * 128:(mt + 1) * 128],
                            ut2s[0][:, mt, cc * 256:(cc + 1) * 256],
                            start=True, stop=False)

            nc.gpsimd.collective_compute(
                "AllGather", mybir.AluOpType.bypass,
                replica_groups=[[0, 1], [2, 3], [4, 5], [6, 7]],
                ins=[cc_in_d[:, :].opt()], outs=[cc_out_d[:, :, :].opt()])
            qhand = cpool.tile([128, 8], DT, tag="qhand")
            nc.sync.dma_start(qhand[:], cc_out_d[0, :, :])

            # broadcast (128,4) -> (128,64) by log-doubling, then rotate
            qrep = []
            for comp in range(2):
                t = cpool.tile([128, NCH * NT], DT, tag=f"qrep{comp}",
                               name=f"qrep{comp}")
                nc.vector.tensor_copy(out=t[:, 0:4],
                                      in_=qhand[:, 4 * comp:4 * comp + 4])
                for w in (4, 8, 16, 32):
                    nc.vector.tensor_copy(out=t[:, w:2 * w], in_=t[:, 0:w])
                qrep.append(t)
            t1 = cpool.tile([128, NCH * NT], DT, tag="d_t1")
            nc.vector.tensor_mul(t1[:], rotC[:], qrep[0][:])
            t2 = cpool.tile([128, NCH * NT], DT, tag="d_t2")
            nc.vector.tensor_mul(t2[:], rotS[:], qrep[1][:])
            Dre = cpool.tile([128, NCH * NT], DT, tag="Dre")
            nc.vector.tensor_sub(Dre[:], t1[:], t2[:])
            t3 = cpool.tile([128, NCH * NT], DT, tag="d_t3")
            nc.vector.tensor_mul(t3[:], rotC[:], qrep[1][:])
            t4 = cpool.tile([128, NCH * NT], DT, tag="d_t4")
            nc.vector.tensor_mul(t4[:], rotS[:], qrep[0][:])
            Dim = cpool.tile([128, NCH * NT], DT, tag="Dim")
            nc.vector.tensor_add(Dim[:], t3[:], t4[:])

            # =================== PHASE 2 ===================
            if True:
                for g in range(NG):
                    if g not in ut2s:
                        ut2s[g] = load_u(g, "ut")
                    if g + 1 < NG and g + 1 not in ut2s:
                        ut2s[g + 1] = load_u(g + 1, "ut")
                    ut2 = ut2s[g]
                    goff = 0
                    p1 = ppool.tile([128, NT, GL], BF, tag="p1")
                    p2 = ppool.tile([128, NT, GL], BF, tag="p2")
                    hT3 = hpool.tile([128, NT, GL], BF, tag="h")
                    for c2 in range(2):
                        cr = slice(c2 * 256, (c2 + 1) * 256)
                        for ntl in range(NT):
                            idx = (2 * g + c2) * NT + ntl
                            nc.vector.scalar_tensor_tensor(
                                p1[:, ntl, cr], W3[g][0][:, ntl, cr],
                                Dre[:, idx:idx + 1], c0T3[:, ntl, cr], add, mult)
                            nc.vector.scalar_tensor_tensor(
                                p2[:, ntl, cr], W3[g][1][:, ntl, cr],
                                Dim[:, idx:idx + 1], s0T3[:, ntl, cr], add, mult)
                        nc.vector.tensor_sub(hT3[:, :, cr], p1[:, :, cr],
                                             p2[:, :, cr])
                    # projection: y_ps (128, 1024) = [mt_local(2) x cc(2) x 256]
                    for mp in range(4):
                        if g == 0:
                            y_ps = y_ps_g0[mp]
                        else:
                            y_ps = psy.tile([128, 2 * GL], DT, tag=f"y{mp}",
                                            name=f"y{mp}_{g}")
                        for half in range(2):
                            mt = 2 * mp + half
                            for cc in range(2):
                                b = half * 512 + cc * 256
                                cr = slice(cc * 256, (cc + 1) * 256)
                                if g != 0:
                                    nc.tensor.matmul(
                                        y_ps[:, b:b + 256],
                                        dD[:, mt * 128:(mt + 1) * 128],
                                        ut2[:, mt, cc * 256:(cc + 1) * 256],
                                        start=True, stop=False)
                                for kt in range(4):
                                    nc.tensor.matmul(
                                        y_ps[:, b:b + 256],
                                        cwT3[:, kt, mt * 128:(mt + 1) * 128],
                                        hT3[:, kt, cr],
                                        start=False, stop=(kt == 3))
                        yo = ypool.tile([128, 2 * GL], BF, tag="yo",
                                        name=f"yo{mp}_{g}")
                        if g == NG - 1 and mp % 2 == 1:
                            nc.vector.tensor_copy(out=yo[:], in_=y_ps[:])
                        else:
                            nc.scalar.copy(yo[:], y_ps[:])
                        nc.sync.dma_start(
                            yT_d[:, 2 * mp:2 * mp + 2, g * GL:(g + 1) * GL],
                            yo[:])
            psy_ctx.__exit__(None, None, None)
    nc.compile()
    return nc


def _host_tables(a_params):
    n = STATE
    half = n // 2
    a_full = np.zeros(n)
    a_full[1:half + 1] = a_params.astype(np.float64)
    a_full[half + 1:] = -a_params.astype(np.float64)[::-1][: n - half - 1]
    omega = np.imag(np.fft.fft(a_full))
    theta = -2.0 * np.arctan(omega)          # (512,)
    sig = np.arange(256)
    cS = np.cos(sig[:, None] * theta[None, :])
    sS = np.sin(sig[:, None] * theta[None, :])
    tabs = {
        "c0Sa": cS[:128], "c0Sb": cS[128:],
        "ms0Sa": -sS[:128], "ms0Sb": -sS[128:],
    }
    tg = np.arange(GL) % 256
    c0T3 = np.empty((128, NT, GL))
    s0T3 = np.empty((128, NT, GL))
    for nt in range(NT):
        th = theta[128 * nt:128 * (nt + 1)]
        c0T3[:, nt, :] = np.cos(th[:, None] * tg[None, :])
        s0T3[:, nt, :] = np.sin(th[:, None] * tg[None, :])
    tabs["c0T3"] = c0T3
    tabs["s0T3"] = s0T3
    thL = theta.reshape(NT, 128).T * L       # (128, NT)
    tabs["cL4"] = np.cos(thL)
    tabs["sL4"] = np.sin(thL)
    # rot tables for the cross-core carry correction: lam^(L*c), c=0..NCH-1
    thP = theta.reshape(NT, 128).T           # (128, NT)
    rotC = np.empty((128, NCH * NT))
    rotS = np.empty((128, NCH * NT))
    for c in range(NCH):
        rotC[:, c * NT:(c + 1) * NT] = np.cos(thP * (L * c))
        rotS[:, c * NT:(c + 1) * NT] = np.sin(thP * (L * c))
    tabs["rotC"] = rotC
    tabs["rotS"] = rotS
    U = np.triu(np.ones((128, 128)))
    tabs["U1"] = np.concatenate([U, np.ones((128, 128))], axis=1)
    return tabs


def kernel(u, a_params, B_w, C_w, D, trace=False):
    u = np.asarray(u, dtype=np.float32)
    B_w = np.asarray(B_w, dtype=np.float32)
    C_w = np.asarray(C_w, dtype=np.float32)
    D = np.asarray(D, dtype=np.float32)
    tabs = _host_tables(np.asarray(a_params))

    if "nc" not in _CACHE:
        _CACHE["nc"] = build_nc()
    nc = _CACHE["nc"]

    bf_tabs = {}
    for k, v in tabs.items():
        dt = np.float32 if k in ("cL4", "sL4", "rotC", "rotS") else BF_NP
        bf_tabs[k] = np.ascontiguousarray(v.astype(dt))
    # pack sigma-rotation tables: csP (128, 4, 512)
    csP = np.stack([bf_tabs.pop("c0Sa"), bf_tabs.pop("c0Sb"),
                    bf_tabs.pop("ms0Sa"), bf_tabs.pop("ms0Sb")], axis=1)
    bf_tabs["csP"] = np.ascontiguousarray(csP)

    # bwT: (128 p, 8 k, 512 ch) with d = k*128 + p
    bwT = np.ascontiguousarray(
        B_w.T.reshape(8, 128, STATE).transpose(1, 0, 2).astype(BF_NP))
    # cwT: (128 p, 4 kt, 1024 d) with ch = kt*128 + p
    cwT = np.ascontiguousarray(
        C_w.T.reshape(4, 128, D_MODEL).transpose(1, 0, 2).astype(BF_NP))
    dD = np.zeros((128, D_MODEL), dtype=BF_NP)
    for mt in range(8):
        blk = np.diag(D[mt * 128:(mt + 1) * 128])
        dD[:, mt * 128:(mt + 1) * 128] = blk.astype(BF_NP)

    in_maps = []
    for core in range(8):
        b, hf = core // 2, core % 2
        # uT: (128 p, 8 k, TH) with d = k*128 + p
        uT = np.ascontiguousarray(
            u[b, hf * TH:(hf + 1) * TH, :].T.reshape(8, 128, TH)
            .transpose(1, 0, 2).astype(BF_NP))
        # fold the half-mask into the correction rot tables
        rotC = np.ascontiguousarray(bf_tabs["rotC"] * float(hf))
        rotS = np.ascontiguousarray(bf_tabs["rotS"] * float(hf))
        m = {"uT": uT, "bwT": bwT, "cwT": cwT, "dD": dD,
             "rotC": rotC, "rotS": rotS}
        for k2, v2 in bf_tabs.items():
            if k2 not in ("rotC", "rotS"):
                m[k2] = v2
        in_maps.append(m)

    res = bass_utils.run_bass_kernel_spmd(
        nc, in_maps, core_ids=list(range(8)), trace=trace)
    y = np.empty((BATCH, SEQ, D_MODEL), dtype=np.float32)
    for core in range(8):
        b, hf = core // 2, core % 2
        yT = np.asarray(res.results[core]["yT"]).astype(np.float32)  # (128,8,TH)
        # y[b, t, mt*128 + p] = yT[p, mt, t]
        y[b, hf * TH:(hf + 1) * TH, :] = yT.transpose(2, 1, 0).reshape(TH, D_MODEL)
    _CACHE["last_res"] = res
    return y


# revision 20
# speedup vs baseline: 1.5526x; 1.0317x over previous
"""Trainium2 Bass kernel for CayleyCirculantSSMLayer (time-split, 2-phase).

Math: lambda_j = (1-i*w_j)/(1+i*w_j) is on the unit circle, so the causal
conv h[t] = sum_{s<=t} Re(lambda^{t-s}) Bu[s] factors through a rotated
cumulative sum chained across 256-row chunks by a per-channel rotation.

Sharding: 8 cores = 4 samples x 2 time-halves. Each core computes Bu +
chunked cumsum for its 4096-step half with a LOCAL carry chain (phase 1,
storing locally-biased accumulators in SBUF as bf16), the cores of a
pair exchange their final carries with one tiny pair-AllGather, then each
core applies the rotated carry correction, combines with the cos/sin
tables and projects to all 1024 output dims (phase 2). The correction is
identically zero on first-half cores (host-provided mask), keeping one
symmetric SPMD program. All matmuls/elementwise run in bf16 (f32 psum).
"""
import sys
import numpy as np
import ml_dtypes

for p in ("/opt/trn_rl_repo",):
    if p not in sys.path:
        sys.path.insert(0, p)

from concourse import bass, bacc, mybir, tile
from concourse import bass_utils

D_MODEL = 1024
STATE = 512
BATCH = 4
SEQ = 8192
TH = SEQ // 2             # per-core time half
L = 256                   # carry-chunk length
GL = 512                  # group length = 2 chunks
NG = TH // GL             # 8 groups per core
NCH = TH // L             # 16 chunks per core
NT = 4                    # state n-tiles of 128
DT = mybir.dt.float32
BF = mybir.dt.bfloat16
BF_NP = ml_dtypes.bfloat16

_CACHE = {}


def build_nc():
    nc = bacc.Bacc(None, target_bir_lowering=False, num_devices=8)
    uT_d = nc.dram_tensor("uT", [128, 8, TH], BF, kind="ExternalInput")
    bwT_d = nc.dram_tensor("bwT", [128, 8, STATE], BF, kind="ExternalInput")
    cwT_d = nc.dram_tensor("cwT", [128, 4, D_MODEL], BF, kind="ExternalInput")
    dD_d = nc.dram_tensor("dD", [128, D_MODEL], BF, kind="ExternalInput")
    csP_d = nc.dram_tensor("csP", [128, 4, STATE], BF, kind="ExternalInput")
    c0T3_d = nc.dram_tensor("c0T3", [128, NT, GL], BF, kind="ExternalInput")
    s0T3_d = nc.dram_tensor("s0T3", [128, NT, GL], BF, kind="ExternalInput")
    cL4_d = nc.dram_tensor("cL4", [128, NT], DT, kind="ExternalInput")
    sL4_d = nc.dram_tensor("sL4", [128, NT], DT, kind="ExternalInput")
    rotC_d = nc.dram_tensor("rotC", [128, NCH * NT], DT, kind="ExternalInput")
    rotS_d = nc.dram_tensor("rotS", [128, NCH * NT], DT, kind="ExternalInput")
    U1_d = nc.dram_tensor("U1", [128, 256], BF, kind="ExternalInput")
    # output layout: [row-in-mt (=partition), mt, time]
    yT_d = nc.dram_tensor("yT", [128, 8, TH], BF, kind="ExternalOutput")

    cc_in_d = nc.dram_tensor("cc_in", [128, 8], DT)
    cc_out_d = nc.dram_tensor("cc_out", [2, 128, 8], DT)

    add = mybir.AluOpType.add
    mult = mybir.AluOpType.mult
    CP = mybir.ActivationFunctionType.Identity

    with tile.TileContext(nc) as tc:
        with (
            tc.tile_pool(name="const", bufs=1) as cpool,
            tc.tile_pool(name="ut", bufs=3) as upool,
            tc.tile_pool(name="bus", bufs=4) as bupool,
            tc.tile_pool(name="v", bufs=4) as vpool,
            tc.tile_pool(name="qc", bufs=2) as qpool,
            tc.tile_pool(name="pgrp", bufs=2) as ppool,
            tc.tile_pool(name="hgrp", bufs=2) as hpool,
            tc.tile_pool(name="yo", bufs=4) as ypool,
        ):
            # ---- phase-1-critical constants first (unblock first Bu fast) --
            bwT3 = cpool.tile([128, 8, STATE], BF, tag="bwT3")
            nc.sync.dma_start(bwT3[:, 0:4, :], bwT_d[:, 0:4, :])
            uts = {}

            def load_u(g, pool_tag, split=False):
                t = upool.tile([128, 8, GL], BF, tag=pool_tag,
                               name=f"{pool_tag}_{g}")
                if split:
                    nc.sync.dma_start(t[:, :, 0:GL // 2],
                                      uT_d[:, :, g * GL:g * GL + GL // 2])
                    nc.sync.dma_start(t[:, :, GL // 2:GL],
                                      uT_d[:, :, g * GL + GL // 2:(g + 1) * GL])
                else:
                    nc.sync.dma_start(
                        t[:], uT_d[:, :, g * GL:(g + 1) * GL])
                return t

            uts[0] = load_u(0, "ut", split=True)
            nc.sync.dma_start(bwT3[:, 4:8, :], bwT_d[:, 4:8, :])
            uts[1] = load_u(1, "ut")
            csPack = cpool.tile([128, 4, STATE], BF, tag="csPack")
            nc.sync.dma_start(csPack[:], csP_d[:, :, :])
            U1 = cpool.tile([128, 256], BF, tag="U1")
            nc.sync.dma_start(U1[:], U1_d[:, :])
            cL4 = cpool.tile([128, NT], DT, tag="cL4")
            nc.sync.dma_start(cL4[:], cL4_d[:, :])
            sL4 = cpool.tile([128, NT], DT, tag="sL4")
            nc.sync.dma_start(sL4[:], sL4_d[:, :])

            # ---- remaining constants (needed later than first Bu) ----
            c0T3 = cpool.tile([128, NT, GL], BF, tag="c0T3")
            nc.sync.dma_start(c0T3[:], c0T3_d[:, :, :])
            s0T3 = cpool.tile([128, NT, GL], BF, tag="s0T3")
            nc.sync.dma_start(s0T3[:], s0T3_d[:, :, :])
            cwT3 = cpool.tile([128, 4, D_MODEL], BF, tag="cwT3")
            nc.sync.dma_start(cwT3[:], cwT_d[:, :, :])

            dD = cpool.tile([128, D_MODEL], BF, tag="dD")
            nc.sync.dma_start(dD[:], dD_d[:, :])
            rotC = cpool.tile([128, NCH * NT], DT, tag="rotC")
            nc.sync.dma_start(rotC[:], rotC_d[:, :])
            rotS = cpool.tile([128, NCH * NT], DT, tag="rotS")
            nc.sync.dma_start(rotS[:], rotS_d[:, :])

            # persistent per-group accumulators (locally-biased), bf16
            W3 = [[cpool.tile([128, NT, GL], BF, tag=f"w3_{g}_{comp}",
                              name=f"w3_{g}_{comp}")
                   for comp in range(2)] for g in range(NG)]

            qre = cpool.tile([128, NT], DT, tag="q0re")
            qim = cpool.tile([128, NT], DT, tag="q0im")
            nc.vector.memset(qre[:], 0.0)
            nc.vector.memset(qim[:], 0.0)
            qsre = cpool.tile([128, NCH * NT], DT, tag="qsre")
            qsim = cpool.tile([128, NCH * NT], DT, tag="qsim")

            # =================== PHASE 1 ===================
            def emit_bu(ut, goff, s4):
                bu_ps = psbu.tile([128, STATE], DT, tag="bu")
                co = goff * GL + s4 * 128
                for k in range(8):
                    nc.tensor.matmul(
                        bu_ps[:], ut[:, k, co:co + 128], bwT3[:, k, :],
                        start=(k == 0), stop=(k == 7))
                buS = bupool.tile([128, STATE], BF, tag="buS")
                nc.vector.tensor_copy(out=buS[:], in_=bu_ps[:])
                vr = vpool.tile([128, STATE], BF, tag="vre")
                nc.vector.tensor_mul(vr[:], buS[:], csPack[:, s4 % 2, :])
                vi = vpool.tile([128, STATE], BF, tag="vim")
                nc.gpsimd.tensor_mul(vi[:, 0:256], buS[:, 0:256],
                                     csPack[:, 2 + s4 % 2, 0:256])
                nc.gpsimd.tensor_mul(vi[:, 256:512], buS[:, 256:512],
                                     csPack[:, 2 + s4 % 2, 256:512])
                return vr, vi

            def emit_chunk(g, c, vre, vim):
                nonlocal qre, qim
                cg = 2 * g + c
                # stash the local carry for this chunk (phase-2 bias)
                nc.vector.tensor_copy(out=qsre[:, cg * NT:(cg + 1) * NT],
                                      in_=qre[:])
                nc.vector.tensor_copy(out=qsim[:, cg * NT:(cg + 1) * NT],
                                      in_=qim[:])
                wlast = []
                for comp, xs in ((0, vre), (1, vim)):
                    wl = qpool.tile([128, NT], DT, tag=f"wl{comp}")
                    for pr in range(2):
                        w_ps = psw.tile([128, GL], DT, tag=f"w{comp}")
                        for half in range(2):
                            ntl = 2 * pr + half
                            b = half * 256
                            x0 = xs[2 * c][:, ntl * 128:(ntl + 1) * 128]
                            x1 = xs[2 * c + 1][:, ntl * 128:(ntl + 1) * 128]
                            nc.tensor.matmul(w_ps[:, b:b + 128], x0,
                                             U1[:, 0:128], start=True, stop=True)
                            nc.tensor.matmul(w_ps[:, b + 128:b + 256], x0,
                                             U1[:, 128:256], start=True, stop=False)
                            nc.tensor.matmul(w_ps[:, b + 128:b + 256], x1,
                                             U1[:, 0:128], start=False, stop=True)
                        # chunk-local sums (pre-bias) for the carry chain
                        nc.vector.tensor_copy(
                            out=wl[:, 2 * pr:2 * pr + 2],
                            in_=w_ps[:, 255:512:256])
                        # unbiased move PSUM -> SBUF (bf16), both ntiles at once
                        nc.scalar.copy(
                            W3[g][comp][:, 2 * pr:2 * pr + 2,
                                        c * 256:(c + 1) * 256],
                            w_ps[:])
                    wlast.append(wl)
                # carry chain: q' = lam^L (Wlast + q)
                tre = qpool.tile([128, NT], DT, tag="tre")
                nc.vector.tensor_add(tre[:], wlast[0][:], qre[:])
                tim = qpool.tile([128, NT], DT, tag="tim")
                nc.vector.tensor_add(tim[:], wlast[1][:], qim[:])
                u1re = qpool.tile([128, NT], DT, tag="u1re")
                nc.vector.tensor_mul(u1re[:], tre[:], cL4[:])
                vv = qpool.tile([128, NT], DT, tag="vv")
                nc.vector.tensor_mul(vv[:], tim[:], sL4[:])
                u1im = qpool.tile([128, NT], DT, tag="u1im")
                nc.vector.tensor_mul(u1im[:], tim[:], cL4[:])
                w2 = qpool.tile([128, NT], DT, tag="w2")
                nc.vector.tensor_mul(w2[:], tre[:], sL4[:])
                qre = qpool.tile([128, NT], DT, tag="qre")
                nc.vector.tensor_sub(qre[:], u1re[:], vv[:])
                qim = qpool.tile([128, NT], DT, tag="qim")
                nc.vector.tensor_add(qim[:], u1im[:], w2[:])

            with (
                tc.tile_pool(name="ps_bu", bufs=2, space="PSUM") as psbu,
                tc.tile_pool(name="ps_w", bufs=3, space="PSUM") as psw,
            ):
                for g in range(NG):
                    if g not in uts:
                        uts[g] = load_u(g, "ut")
                    if g + 1 < NG and g + 1 not in uts:
                        uts[g + 1] = load_u(g + 1, "ut")
                    ut = uts[g]
                    goff = 0
                    vre, vim = [], []
                    for s4 in range(4):
                        vr, vi = emit_bu(ut, goff, s4)
                        vre.append(vr)
                        vim.append(vi)
                    emit_chunk(g, 0, vre, vim)
                    emit_chunk(g, 1, vre, vim)

            # =================== CARRY EXCHANGE ===================
            psy_ctx = tc.tile_pool(name="ps_y", bufs=1, space="PSUM")
            psy = psy_ctx.__enter__()
            y_ps_g0 = {}
            qpack = cpool.tile([128, 8], DT, tag="qpack")
            nc.gpsimd.tensor_copy(out=qpack[:, 0:4], in_=qre[:])
            nc.gpsimd.tensor_copy(out=qpack[:, 4:8], in_=qim[:])
            nc.sync.dma_start(cc_in_d[:, :], qpack[:])

            # prefetch first phase-2 u groups (independent of collective)
            ut2s = {0: load_u(0, "ut"), 1: load_u(1, "ut")}
            # pre-run g0's skip matmuls into PSUM during the collective window
            PRERUN = False
            for mp in range(4):
                if not PRERUN:
                    break
                y_ps = psy.tile([128, 2 * GL], DT, tag=f"y{mp}",
                                name=f"y{mp}_pre")
                y_ps_g0[mp] = y_ps
                for half in range(2):
                    mt = 2 * mp + half
                    for cc in range(2):
                        b = half * 512 + cc * 256
                        nc.tensor.matmul(
                            y_ps[:, b:b + 256],
                            dD[:, mt * 128:(mt + 1) * 128],
                            ut2s[0][:, mt, cc * 256:(cc + 1) * 256],
                            start=True, stop=False)

            nc.gpsimd.collective_compute(
                "AllGather", mybir.AluOpType.bypass,
                replica_groups=[[0, 1], [2, 3], [4, 5], [6, 7]],
                ins=[cc_in_d[:, :].opt()], outs=[cc_out_d[:, :, :].opt()])
            qhand = cpool.tile([128, 8], DT, tag="qhand")
            nc.sync.dma_start(qhand[:], cc_out_d[0, :, :])

            # broadcast (128,4) -> (128,64) by log-doubling, then rotate
            qrep = []
            for comp in range(2):
                t = cpool.tile([128, NCH * NT], DT, tag=f"qrep{comp}",
                               name=f"qrep{comp}")
                nc.vector.tensor_copy(out=t[:, 0:4],
                                      in_=qhand[:, 4 * comp:4 * comp + 4])
                for w in (4, 8, 16, 32):
                    nc.vector.tensor_copy(out=t[:, w:2 * w], in_=t[:, 0:w])
                qrep.append(t)
            t1 = cpool.tile([128, NCH * NT], DT, tag="d_t1")
            nc.vector.tensor_mul(t1[:], rotC[:], qrep[0][:])
            t2 = cpool.tile([128, NCH * NT], DT, tag="d_t2")
            nc.vector.tensor_mul(t2[:], rotS[:], qrep[1][:])
            Dre = cpool.tile([128, NCH * NT], DT, tag="Dre")
            nc.vector.tensor_sub(Dre[:], t1[:], t2[:])
            t3 = cpool.tile([128, NCH * NT], DT, tag="d_t3")
            nc.vector.tensor_mul(t3[:], rotC[:], qrep[1][:])
            t4 = cpool.tile([128, NCH * NT], DT, tag="d_t4")
            nc.vector.tensor_mul(t4[:], rotS[:], qrep[0][:])
            Dim0 = cpool.tile([128, NCH * NT], DT, tag="Dim0")
            nc.vector.tensor_add(Dim0[:], t3[:], t4[:])
            DreT = cpool.tile([128, NCH * NT], DT, tag="DreT")
            nc.vector.tensor_add(DreT[:], Dre[:], qsre[:])
            Dim = cpool.tile([128, NCH * NT], DT, tag="DimT")
            nc.vector.tensor_add(Dim[:], Dim0[:], qsim[:])
            Dre = DreT

            # =================== PHASE 2 ===================
            if True:
                for g in range(NG):
                    if g not in ut2s:
                        ut2s[g] = load_u(g, "ut")
                    if g + 1 < NG and g + 1 not in ut2s:
                        ut2s[g + 1] = load_u(g + 1, "ut")
                    ut2 = ut2s[g]
                    goff = 0
                    p1 = ppool.tile([128, NT, GL], BF, tag="p1")
                    p2 = ppool.tile([128, NT, GL], BF, tag="p2")
                    hT3 = hpool.tile([128, NT, GL], BF, tag="h")
                    for c2 in range(2):
                        cr = slice(c2 * 256, (c2 + 1) * 256)
                        for ntl in range(NT):
                            idx = (2 * g + c2) * NT + ntl
                            nc.vector.scalar_tensor_tensor(
                                p1[:, ntl, cr], W3[g][0][:, ntl, cr],
                                Dre[:, idx:idx + 1], c0T3[:, ntl, cr], add, mult)
                            nc.vector.scalar_tensor_tensor(
                                p2[:, ntl, cr], W3[g][1][:, ntl, cr],
                                Dim[:, idx:idx + 1], s0T3[:, ntl, cr], add, mult)
                        nc.vector.tensor_sub(hT3[:, :, cr], p1[:, :, cr],
                                             p2[:, :, cr])
                    # projection: y_ps (128, 1024) = [mt_local(2) x cc(2) x 256]
                    for mp in range(4):
                        if g == 0 and y_ps_g0:
                            y_ps = y_ps_g0[mp]
                        else:
                            y_ps = psy.tile([128, 2 * GL], DT, tag=f"y{mp}",
                                            name=f"y{mp}_{g}")
                        for half in range(2):
                            mt = 2 * mp + half
                            for cc in range(2):
                                b = half * 512 + cc * 256
                                cr = slice(cc * 256, (cc + 1) * 256)
                                if g != 0 or not y_ps_g0:
                                    nc.tensor.matmul(
                                        y_ps[:, b:b + 256],
                                        dD[:, mt * 128:(mt + 1) * 128],
                                        ut2[:, mt, cc * 256:(cc + 1) * 256],
                                        start=True, stop=False)
                                for kt in range(4):
                                    nc.tensor.matmul(
                                        y_ps[:, b:b + 256],
                                        cwT3[:, kt, mt * 128:(mt + 1) * 128],
                                        hT3[:, kt, cr],
                                        start=False, stop=(kt == 3))
                        yo = ypool.tile([128, 2 * GL], BF, tag="yo",
                                        name=f"yo{mp}_{g}")
                        if g == NG - 1 and mp % 2 == 1:
                            nc.vector.tensor_copy(out=yo[:], in_=y_ps[:])
                        else:
                            nc.scalar.copy(yo[:], y_ps[:])
                        nc.sync.dma_start(
                            yT_d[:, 2 * mp:2 * mp + 2, g * GL:(g + 1) * GL],
                            yo[:])
            psy_ctx.__exit__(None, None, None)
    nc.compile()
    return nc


def _host_tables(a_params):
    n = STATE
    half = n // 2
    a_full = np.zeros(n)
    a_full[1:half + 1] = a_params.astype(np.float64)
    a_full[half + 1:] = -a_params.astype(np.float64)[::-1][: n - half - 1]
    omega = np.imag(np.fft.fft(a_full))
    theta = -2.0 * np.arctan(omega)          # (512,)
    sig = np.arange(256)
    cS = np.cos(sig[:, None] * theta[None, :])
    sS = np.sin(sig[:, None] * theta[None, :])
    tabs = {
        "c0Sa": cS[:128], "c0Sb": cS[128:],
        "ms0Sa": -sS[:128], "ms0Sb": -sS[128:],
    }
    tg = np.arange(GL) % 256
    c0T3 = np.empty((128, NT, GL))
    s0T3 = np.empty((128, NT, GL))
    for nt in range(NT):
        th = theta[128 * nt:128 * (nt + 1)]
        c0T3[:, nt, :] = np.cos(th[:, None] * tg[None, :])
        s0T3[:, nt, :] = np.sin(th[:, None] * tg[None, :])
    tabs["c0T3"] = c0T3
    tabs["s0T3"] = s0T3
    thL = theta.reshape(NT, 128).T * L       # (128, NT)
    tabs["cL4"] = np.cos(thL)
    tabs["sL4"] = np.sin(thL)
    # rot tables for the cross-core carry correction: lam^(L*c), c=0..NCH-1
    thP = theta.reshape(NT, 128).T           # (128, NT)
    rotC = np.empty((128, NCH * NT))
    rotS = np.empty((128, NCH * NT))
    for c in range(NCH):
        rotC[:, c * NT:(c + 1) * NT] = np.cos(thP * (L * c))
        rotS[:, c * NT:(c + 1) * NT] = np.sin(thP * (L * c))
    tabs["rotC"] = rotC
    tabs["rotS"] = rotS
    U = np.triu(np.ones((128, 128)))
    tabs["U1"] = np.concatenate([U, np.ones((128, 128))], axis=1)
    return tabs


def kernel(u, a_params, B_w, C_w, D, trace=False):
    u = np.asarray(u, dtype=np.float32)
    B_w = np.asarray(B_w, dtype=np.float32)
    C_w = np.asarray(C_w, dtype=np.float32)
    D = np.asarray(D, dtype=np.float32)
    tabs = _host_tables(np.asarray(a_params))

    if "nc" not in _CACHE:
        _CACHE["nc"] = build_nc()
    nc = _CACHE["nc"]

    bf_tabs = {}
    for k, v in tabs.items():
        dt = np.float32 if k in ("cL4", "sL4", "rotC", "rotS") else BF_NP
        bf_tabs[k] = np.ascontiguousarray(v.astype(dt))
    # pack sigma-rotation tables: csP (128, 4, 512)
    csP = np.stack([bf_tabs.pop("c0Sa"), bf_tabs.pop("c0Sb"),
                    bf_tabs.pop("ms0Sa"), bf_tabs.pop("ms0Sb")], axis=1)
    bf_tabs["csP"] = np.ascontiguousarray(csP)

    # bwT: (128 p, 8 k, 512 ch) with d = k*128 + p
    bwT = np.ascontiguousarray(
        B_w.T.reshape(8, 128, STATE).transpose(1, 0, 2).astype(BF_NP))
    # cwT: (128 p, 4 kt, 1024 d) with ch = kt*128 + p
    cwT = np.ascontiguousarray(
        C_w.T.reshape(4, 128, D_MODEL).transpose(1, 0, 2).astype(BF_NP))
    dD = np.zeros((128, D_MODEL), dtype=BF_NP)
    for mt in range(8):
        blk = np.diag(D[mt * 128:(mt + 1) * 128])
        dD[:, mt * 128:(mt + 1) * 128] = blk.astype(BF_NP)

    in_maps = []
    for core in range(8):
        b, hf = core // 2, core % 2
        # uT: (128 p, 8 k, TH) with d = k*128 + p
        uT = np.ascontiguousarray(
            u[b, hf * TH:(hf + 1) * TH, :].T.reshape(8, 128, TH)
            .transpose(1, 0, 2).astype(BF_NP))
        # fold the half-mask into the correction rot tables
        rotC = np.ascontiguousarray(bf_tabs["rotC"] * float(hf))
        rotS = np.ascontiguousarray(bf_tabs["rotS"] * float(hf))
        m = {"uT": uT, "bwT": bwT, "cwT": cwT, "dD": dD,
             "rotC": rotC, "rotS": rotS}
        for k2, v2 in bf_tabs.items():
            if k2 not in ("rotC", "rotS"):
                m[k2] = v2
        in_maps.append(m)

    res = bass_utils.run_bass_kernel_spmd(
        nc, in_maps, core_ids=list(range(8)), trace=trace)
    y = np.empty((BATCH, SEQ, D_MODEL), dtype=np.float32)
    for core in range(8):
        b, hf = core // 2, core % 2
        yT = np.asarray(res.results[core]["yT"]).astype(np.float32)  # (128,8,TH)
        # y[b, t, mt*128 + p] = yT[p, mt, t]
        y[b, hf * TH:(hf + 1) * TH, :] = yT.transpose(2, 1, 0).reshape(TH, D_MODEL)
    _CACHE["last_res"] = res
    return y


# revision 24
# speedup vs baseline: 1.5624x; 1.0063x over previous
"""Trainium2 Bass kernel for CayleyCirculantSSMLayer (time-split, 2-phase).

Math: lambda_j = (1-i*w_j)/(1+i*w_j) is on the unit circle, so the causal
conv h[t] = sum_{s<=t} Re(lambda^{t-s}) Bu[s] factors through a rotated
cumulative sum chained across 256-row chunks by a per-channel rotation.

Sharding: 8 cores = 4 samples x 2 time-halves. Each core computes Bu +
chunked cumsum for its 4096-step half with a LOCAL carry chain (phase 1,
storing locally-biased accumulators in SBUF as bf16), the cores of a
pair exchange their final carries with one tiny pair-AllGather, then each
core applies the rotated carry correction, combines with the cos/sin
tables and projects to all 1024 output dims (phase 2). The correction is
identically zero on first-half cores (host-provided mask), keeping one
symmetric SPMD program. All matmuls/elementwise run in bf16 (f32 psum).
"""
import sys
import numpy as np
import ml_dtypes

for p in ("/opt/trn_rl_repo",):
    if p not in sys.path:
        sys.path.insert(0, p)

from concourse import bass, bacc, mybir, tile
from concourse import bass_utils

D_MODEL = 1024
STATE = 512
BATCH = 4
SEQ = 8192
TH = SEQ // 2             # per-core time half
L = 256                   # carry-chunk length
GL = 512                  # group length = 2 chunks
NG = TH // GL             # 8 groups per core
NCH = TH // L             # 16 chunks per core
NT = 4                    # state n-tiles of 128
DT = mybir.dt.float32
BF = mybir.dt.bfloat16
BF_NP = ml_dtypes.bfloat16

_CACHE = {}


def build_nc():
    nc = bacc.Bacc(None, target_bir_lowering=False, num_devices=8)
    uT_d = nc.dram_tensor("uT", [128, 8, TH], BF, kind="ExternalInput")
    bwT_d = nc.dram_tensor("bwT", [128, 8, STATE], BF, kind="ExternalInput")
    cwT_d = nc.dram_tensor("cwT", [128, 4, D_MODEL], BF, kind="ExternalInput")
    dD_d = nc.dram_tensor("dD", [128, D_MODEL], BF, kind="ExternalInput")
    csP_d = nc.dram_tensor("csP", [128, 4, STATE], BF, kind="ExternalInput")
    c0T3_d = nc.dram_tensor("c0T3", [128, NT, GL], BF, kind="ExternalInput")
    s0T3_d = nc.dram_tensor("s0T3", [128, NT, GL], BF, kind="ExternalInput")
    cL4_d = nc.dram_tensor("cL4", [128, NT], DT, kind="ExternalInput")
    sL4_d = nc.dram_tensor("sL4", [128, NT], DT, kind="ExternalInput")
    rotC_d = nc.dram_tensor("rotC", [128, NCH * NT], DT, kind="ExternalInput")
    rotS_d = nc.dram_tensor("rotS", [128, NCH * NT], DT, kind="ExternalInput")
    U1_d = nc.dram_tensor("U1", [128, 256], BF, kind="ExternalInput")
    # output layout: [row-in-mt (=partition), mt, time]
    yT_d = nc.dram_tensor("yT", [128, 8, TH], BF, kind="ExternalOutput")

    cc_in_d = nc.dram_tensor("cc_in", [128, 8], DT)
    cc_out_d = nc.dram_tensor("cc_out", [2, 128, 8], DT)

    add = mybir.AluOpType.add
    mult = mybir.AluOpType.mult
    CP = mybir.ActivationFunctionType.Identity

    with tile.TileContext(nc) as tc:
        with (
            tc.tile_pool(name="const", bufs=1) as cpool,
            tc.tile_pool(name="ut", bufs=3) as upool,
            tc.tile_pool(name="bus", bufs=4) as bupool,
            tc.tile_pool(name="v", bufs=4) as vpool,
            tc.tile_pool(name="qc", bufs=2) as qpool,
            tc.tile_pool(name="pgrp", bufs=2) as ppool,
            tc.tile_pool(name="hgrp", bufs=2) as hpool,
            tc.tile_pool(name="yo", bufs=4) as ypool,
        ):
            # ---- phase-1-critical constants first (unblock first Bu fast) --
            bwT3 = cpool.tile([128, 8, STATE], BF, tag="bwT3")
            nc.sync.dma_start(bwT3[:, 0:4, :], bwT_d[:, 0:4, :])
            uts = {}

            def load_u(g, pool_tag, split=False):
                t = upool.tile([128, 8, GL], BF, tag=pool_tag,
                               name=f"{pool_tag}_{g}")
                if split:
                    nc.sync.dma_start(t[:, :, 0:GL // 2],
                                      uT_d[:, :, g * GL:g * GL + GL // 2])
                    nc.sync.dma_start(t[:, :, GL // 2:GL],
                                      uT_d[:, :, g * GL + GL // 2:(g + 1) * GL])
                else:
                    nc.sync.dma_start(
                        t[:], uT_d[:, :, g * GL:(g + 1) * GL])
                return t

            uts[0] = load_u(0, "ut", split=True)
            nc.sync.dma_start(bwT3[:, 4:8, :], bwT_d[:, 4:8, :])
            uts[1] = load_u(1, "ut")
            csPack = cpool.tile([128, 4, STATE], BF, tag="csPack")
            nc.sync.dma_start(csPack[:], csP_d[:, :, :])
            U1 = cpool.tile([128, 256], BF, tag="U1")
            nc.sync.dma_start(U1[:], U1_d[:, :])
            cL4 = cpool.tile([128, NT], DT, tag="cL4")
            nc.sync.dma_start(cL4[:], cL4_d[:, :])
            sL4 = cpool.tile([128, NT], DT, tag="sL4")
            nc.sync.dma_start(sL4[:], sL4_d[:, :])

            # ---- remaining constants (needed later than first Bu) ----
            c0T3 = cpool.tile([128, NT, GL], BF, tag="c0T3")
            nc.sync.dma_start(c0T3[:], c0T3_d[:, :, :])
            s0T3 = cpool.tile([128, NT, GL], BF, tag="s0T3")
            nc.sync.dma_start(s0T3[:], s0T3_d[:, :, :])
            cwT3 = cpool.tile([128, 4, D_MODEL], BF, tag="cwT3")
            nc.sync.dma_start(cwT3[:], cwT_d[:, :, :])

            dD = cpool.tile([128, D_MODEL], BF, tag="dD")
            nc.sync.dma_start(dD[:], dD_d[:, :])
            rotC = cpool.tile([128, NCH * NT], DT, tag="rotC")
            nc.sync.dma_start(rotC[:], rotC_d[:, :])
            rotS = cpool.tile([128, NCH * NT], DT, tag="rotS")
            nc.sync.dma_start(rotS[:], rotS_d[:, :])

            # persistent per-group accumulators (locally-biased), bf16
            W3 = [[cpool.tile([128, NT, GL], BF, tag=f"w3_{g}_{comp}",
                              name=f"w3_{g}_{comp}")
                   for comp in range(2)] for g in range(NG)]

            qre = cpool.tile([128, NT], DT, tag="q0re")
            qim = cpool.tile([128, NT], DT, tag="q0im")
            nc.vector.memset(qre[:], 0.0)
            nc.vector.memset(qim[:], 0.0)
            qsre = cpool.tile([128, NCH * NT], DT, tag="qsre")
            qsim = cpool.tile([128, NCH * NT], DT, tag="qsim")

            # =================== PHASE 1 ===================
            def emit_bu(ut, goff, s4):
                bu_ps = psbu.tile([128, STATE], DT, tag="bu")
                co = goff * GL + s4 * 128
                for k in range(8):
                    nc.tensor.matmul(
                        bu_ps[:], ut[:, k, co:co + 128], bwT3[:, k, :],
                        start=(k == 0), stop=(k == 7))
                buS = bupool.tile([128, STATE], BF, tag="buS")
                nc.vector.tensor_copy(out=buS[:], in_=bu_ps[:])
                vr = vpool.tile([128, STATE], BF, tag="vre")
                nc.vector.tensor_mul(vr[:], buS[:], csPack[:, s4 % 2, :])
                vi = vpool.tile([128, STATE], BF, tag="vim")
                nc.gpsimd.tensor_mul(vi[:, 0:256], buS[:, 0:256],
                                     csPack[:, 2 + s4 % 2, 0:256])
                nc.gpsimd.tensor_mul(vi[:, 256:512], buS[:, 256:512],
                                     csPack[:, 2 + s4 % 2, 256:512])
                return vr, vi

            def emit_chunk(g, c, vre, vim):
                nonlocal qre, qim
                cg = 2 * g + c
                # stash the local carry for this chunk (phase-2 bias)
                nc.vector.tensor_copy(out=qsre[:, cg * NT:(cg + 1) * NT],
                                      in_=qre[:])
                nc.vector.tensor_copy(out=qsim[:, cg * NT:(cg + 1) * NT],
                                      in_=qim[:])
                wlast = []
                for comp, xs in ((0, vre), (1, vim)):
                    wl = qpool.tile([128, NT], DT, tag=f"wl{comp}")
                    w_ps = psw.tile([128, 2 * GL], DT, tag="w")
                    for ntl in range(NT):
                        b = ntl * 256
                        x0 = xs[2 * c][:, ntl * 128:(ntl + 1) * 128]
                        x1 = xs[2 * c + 1][:, ntl * 128:(ntl + 1) * 128]
                        nc.tensor.matmul(w_ps[:, b:b + 128], x0,
                                         U1[:, 0:128], start=True, stop=True)
                        nc.tensor.matmul(w_ps[:, b + 128:b + 256], x0,
                                         U1[:, 128:256], start=True, stop=False)
                        nc.tensor.matmul(w_ps[:, b + 128:b + 256], x1,
                                         U1[:, 0:128], start=False, stop=True)
                    # chunk-local sums (pre-bias) for the carry chain
                    nc.vector.tensor_copy(
                        out=wl[:], in_=w_ps[:, 255:1024:256])
                    # unbiased move PSUM -> SBUF (bf16), all 4 ntiles at once
                    nc.scalar.copy(
                        W3[g][comp][:, :, c * 256:(c + 1) * 256],
                        w_ps[:])
                    wlast.append(wl)
                # carry chain: q' = lam^L (Wlast + q)
                tre = qpool.tile([128, NT], DT, tag="tre")
                nc.vector.tensor_add(tre[:], wlast[0][:], qre[:])
                tim = qpool.tile([128, NT], DT, tag="tim")
                nc.vector.tensor_add(tim[:], wlast[1][:], qim[:])
                u1re = qpool.tile([128, NT], DT, tag="u1re")
                nc.vector.tensor_mul(u1re[:], tre[:], cL4[:])
                vv = qpool.tile([128, NT], DT, tag="vv")
                nc.vector.tensor_mul(vv[:], tim[:], sL4[:])
                u1im = qpool.tile([128, NT], DT, tag="u1im")
                nc.vector.tensor_mul(u1im[:], tim[:], cL4[:])
                w2 = qpool.tile([128, NT], DT, tag="w2")
                nc.vector.tensor_mul(w2[:], tre[:], sL4[:])
                qre = qpool.tile([128, NT], DT, tag="qre")
                nc.vector.tensor_sub(qre[:], u1re[:], vv[:])
                qim = qpool.tile([128, NT], DT, tag="qim")
                nc.vector.tensor_add(qim[:], u1im[:], w2[:])

            with (
                tc.tile_pool(name="ps_bu", bufs=2, space="PSUM") as psbu,
                tc.tile_pool(name="ps_w", bufs=3, space="PSUM") as psw,
            ):
                for g in range(NG):
                    if g not in uts:
                        uts[g] = load_u(g, "ut")
                    if g + 1 < NG and g + 1 not in uts:
                        uts[g + 1] = load_u(g + 1, "ut")
                    ut = uts[g]
                    goff = 0
                    vre, vim = [], []
                    for s4 in range(4):
                        vr, vi = emit_bu(ut, goff, s4)
                        vre.append(vr)
                        vim.append(vi)
                    emit_chunk(g, 0, vre, vim)
                    emit_chunk(g, 1, vre, vim)

            # =================== CARRY EXCHANGE ===================
            psy_ctx = tc.tile_pool(name="ps_y", bufs=1, space="PSUM")
            psy = psy_ctx.__enter__()
            y_ps_g0 = {}
            qpack = cpool.tile([128, 8], DT, tag="qpack")
            nc.gpsimd.tensor_copy(out=qpack[:, 0:4], in_=qre[:])
            nc.gpsimd.tensor_copy(out=qpack[:, 4:8], in_=qim[:])
            nc.sync.dma_start(cc_in_d[:, :], qpack[:])

            # prefetch first phase-2 u groups (independent of collective)
            ut2s = {0: load_u(0, "ut"), 1: load_u(1, "ut")}
            # pre-run g0's skip matmuls into PSUM during the collective window
            PRERUN = False
            for mp in range(4):
                if not PRERUN:
                    break
                y_ps = psy.tile([128, 2 * GL], DT, tag=f"y{mp}",
                                name=f"y{mp}_pre")
                y_ps_g0[mp] = y_ps
                for half in range(2):
                    mt = 2 * mp + half
                    for cc in range(2):
                        b = half * 512 + cc * 256
                        nc.tensor.matmul(
                            y_ps[:, b:b + 256],
                            dD[:, mt * 128:(mt + 1) * 128],
                            ut2s[0][:, mt, cc * 256:(cc + 1) * 256],
                            start=True, stop=False)

            nc.gpsimd.collective_compute(
                "AllGather", mybir.AluOpType.bypass,
                replica_groups=[[0, 1], [2, 3], [4, 5], [6, 7]],
                ins=[cc_in_d[:, :].opt()], outs=[cc_out_d[:, :, :].opt()])
            qhand = cpool.tile([128, 8], DT, tag="qhand")
            nc.sync.dma_start(qhand[:], cc_out_d[0, :, :])

            # broadcast (128,4) -> (128,64) by log-doubling, then rotate
            qrep = []
            for comp in range(2):
                t = cpool.tile([128, NCH * NT], DT, tag=f"qrep{comp}",
                               name=f"qrep{comp}")
                nc.vector.tensor_copy(out=t[:, 0:4],
                                      in_=qhand[:, 4 * comp:4 * comp + 4])
                for w in (4, 8, 16, 32):
                    nc.vector.tensor_copy(out=t[:, w:2 * w], in_=t[:, 0:w])
                qrep.append(t)
            t1 = cpool.tile([128, NCH * NT], DT, tag="d_t1")
            nc.vector.tensor_mul(t1[:], rotC[:], qrep[0][:])
            t2 = cpool.tile([128, NCH * NT], DT, tag="d_t2")
            nc.vector.tensor_mul(t2[:], rotS[:], qrep[1][:])
            Dre = cpool.tile([128, NCH * NT], DT, tag="Dre")
            nc.vector.tensor_sub(Dre[:], t1[:], t2[:])
            t3 = cpool.tile([128, NCH * NT], DT, tag="d_t3")
            nc.vector.tensor_mul(t3[:], rotC[:], qrep[1][:])
            t4 = cpool.tile([128, NCH * NT], DT, tag="d_t4")
            nc.vector.tensor_mul(t4[:], rotS[:], qrep[0][:])
            Dim0 = cpool.tile([128, NCH * NT], DT, tag="Dim0")
            nc.vector.tensor_add(Dim0[:], t3[:], t4[:])
            DreT = cpool.tile([128, NCH * NT], DT, tag="DreT")
            nc.vector.tensor_add(DreT[:], Dre[:], qsre[:])
            Dim = cpool.tile([128, NCH * NT], DT, tag="DimT")
            nc.vector.tensor_add(Dim[:], Dim0[:], qsim[:])
            Dre = DreT

            # =================== PHASE 2 ===================
            if True:
                for g in range(NG):
                    if g not in ut2s:
                        ut2s[g] = load_u(g, "ut")
                    if g + 1 < NG and g + 1 not in ut2s:
                        ut2s[g + 1] = load_u(g + 1, "ut")
                    ut2 = ut2s[g]
                    goff = 0
                    p1 = ppool.tile([128, NT, GL], BF, tag="p1")
                    p2 = ppool.tile([128, NT, GL], BF, tag="p2")
                    hT3 = hpool.tile([128, NT, GL], BF, tag="h")
                    for c2 in range(2):
                        cr = slice(c2 * 256, (c2 + 1) * 256)
                        for ntl in range(NT):
                            idx = (2 * g + c2) * NT + ntl
                            nc.vector.scalar_tensor_tensor(
                                p1[:, ntl, cr], W3[g][0][:, ntl, cr],
                                Dre[:, idx:idx + 1], c0T3[:, ntl, cr], add, mult)
                            nc.vector.scalar_tensor_tensor(
                                p2[:, ntl, cr], W3[g][1][:, ntl, cr],
                                Dim[:, idx:idx + 1], s0T3[:, ntl, cr], add, mult)
                        nc.vector.tensor_sub(hT3[:, :, cr], p1[:, :, cr],
                                             p2[:, :, cr])
                    # projection: y_ps (128, 1024) = [mt_local(2) x cc(2) x 256]
                    for mp in range(4):
                        if g == 0 and y_ps_g0:
                            y_ps = y_ps_g0[mp]
                        else:
                            y_ps = psy.tile([128, 2 * GL], DT, tag=f"y{mp}",
                                            name=f"y{mp}_{g}")
                        for half in range(2):
                            mt = 2 * mp + half
                            for cc in range(2):
                                b = half * 512 + cc * 256
                                cr = slice(cc * 256, (cc + 1) * 256)
                                if g != 0 or not y_ps_g0:
                                    nc.tensor.matmul(
                                        y_ps[:, b:b + 256],
                                        dD[:, mt * 128:(mt + 1) * 128],
                                        ut2[:, mt, cc * 256:(cc + 1) * 256],
                                        start=True, stop=False)
                                for kt in range(4):
                                    nc.tensor.matmul(
                                        y_ps[:, b:b + 256],
                                        cwT3[:, kt, mt * 128:(mt + 1) * 128],
                                        hT3[:, kt, cr],
                                        start=False, stop=(kt == 3))
                        yo = ypool.tile([128, 2 * GL], BF, tag="yo",
                                        name=f"yo{mp}_{g}")
                        if g == NG - 1 and mp % 2 == 1:
                            nc.vector.tensor_copy(out=yo[:], in_=y_ps[:])
                        else:
                            nc.scalar.copy(yo[:], y_ps[:])
                        nc.sync.dma_start(
                            yT_d[:, 2 * mp:2 * mp + 2, g * GL:(g + 1) * GL],
                            yo[:])
            psy_ctx.__exit__(None, None, None)
    nc.compile()
    return nc


def _host_tables(a_params):
    n = STATE
    half = n // 2
    a_full = np.zeros(n)
    a_full[1:half + 1] = a_params.astype(np.float64)
    a_full[half + 1:] = -a_params.astype(np.float64)[::-1][: n - half - 1]
    omega = np.imag(np.fft.fft(a_full))
    theta = -2.0 * np.arctan(omega)          # (512,)
    sig = np.arange(256)
    cS = np.cos(sig[:, None] * theta[None, :])
    sS = np.sin(sig[:, None] * theta[None, :])
    tabs = {
        "c0Sa": cS[:128], "c0Sb": cS[128:],
        "ms0Sa": -sS[:128], "ms0Sb": -sS[128:],
    }
    tg = np.arange(GL) % 256
    c0T3 = np.empty((128, NT, GL))
    s0T3 = np.empty((128, NT, GL))
    for nt in range(NT):
        th = theta[128 * nt:128 * (nt + 1)]
        c0T3[:, nt, :] = np.cos(th[:, None] * tg[None, :])
        s0T3[:, nt, :] = np.sin(th[:, None] * tg[None, :])
    tabs["c0T3"] = c0T3
    tabs["s0T3"] = s0T3
    thL = theta.reshape(NT, 128).T * L       # (128, NT)
    tabs["cL4"] = np.cos(thL)
    tabs["sL4"] = np.sin(thL)
    # rot tables for the cross-core carry correction: lam^(L*c), c=0..NCH-1
    thP = theta.reshape(NT, 128).T           # (128, NT)
    rotC = np.empty((128, NCH * NT))
    rotS = np.empty((128, NCH * NT))
    for c in range(NCH):
        rotC[:, c * NT:(c + 1) * NT] = np.cos(thP * (L * c))
        rotS[:, c * NT:(c + 1) * NT] = np.sin(thP * (L * c))
    tabs["rotC"] = rotC
    tabs["rotS"] = rotS
    U = np.triu(np.ones((128, 128)))
    tabs["U1"] = np.concatenate([U, np.ones((128, 128))], axis=1)
    return tabs


def kernel(u, a_params, B_w, C_w, D, trace=False):
    u = np.asarray(u, dtype=np.float32)
    B_w = np.asarray(B_w, dtype=np.float32)
    C_w = np.asarray(C_w, dtype=np.float32)
    D = np.asarray(D, dtype=np.float32)
    tabs = _host_tables(np.asarray(a_params))

    if "nc" not in _CACHE:
        _CACHE["nc"] = build_nc()
    nc = _CACHE["nc"]

    bf_tabs = {}
    for k, v in tabs.items():
        dt = np.float32 if k in ("cL4", "sL4", "rotC", "rotS") else BF_NP
        bf_tabs[k] = np.ascontiguousarray(v.astype(dt))
    # pack sigma-rotation tables: csP (128, 4, 512)
    csP = np.stack([bf_tabs.pop("c0Sa"), bf_tabs.pop("c0Sb"),
                    bf_tabs.pop("ms0Sa"), bf_tabs.pop("ms0Sb")], axis=1)
    bf_tabs["csP"] = np.ascontiguousarray(csP)

    # bwT: (128 p, 8 k, 512 ch) with d = k*128 + p
    bwT = np.ascontiguousarray(
        B_w.T.reshape(8, 128, STATE).transpose(1, 0, 2).astype(BF_NP))
    # cwT: (128 p, 4 kt, 1024 d) with ch = kt*128 + p
    cwT = np.ascontiguousarray(
        C_w.T.reshape(4, 128, D_MODEL).transpose(1, 0, 2).astype(BF_NP))
    dD = np.zeros((128, D_MODEL), dtype=BF_NP)
    for mt in range(8):
        blk = np.diag(D[mt * 128:(mt + 1) * 128])
        dD[:, mt * 128:(mt + 1) * 128] = blk.astype(BF_NP)

    in_maps = []
    for core in range(8):
        b, hf = core // 2, core % 2
        # uT: (128 p, 8 k, TH) with d = k*128 + p
        uT = np.ascontiguousarray(
            u[b, hf * TH:(hf + 1) * TH, :].T.reshape(8, 128, TH)
            .transpose(1, 0, 2).astype(BF_NP))
        # fold the half-mask into the correction rot tables
        rotC = np.ascontiguousarray(bf_tabs["rotC"] * float(hf))
        rotS = np.ascontiguousarray(bf_tabs["rotS"] * float(hf))
        m = {"uT": uT, "bwT": bwT, "cwT": cwT, "dD": dD,
             "rotC": rotC, "rotS": rotS}
        for k2, v2 in bf_tabs.items():
            if k2 not in ("rotC", "rotS"):
                m[k2] = v2
        in_maps.append(m)

    res = bass_utils.run_bass_kernel_spmd(
        nc, in_maps, core_ids=list(range(8)), trace=trace)
    y = np.empty((BATCH, SEQ, D_MODEL), dtype=np.float32)
    for core in range(8):
        b, hf = core // 2, core % 2
        yT = np.asarray(res.results[core]["yT"]).astype(np.float32)  # (128,8,TH)
        # y[b, t, mt*128 + p] = yT[p, mt, t]
        y[b, hf * TH:(hf + 1) * TH, :] = yT.transpose(2, 1, 0).reshape(TH, D_MODEL)
    _CACHE["last_res"] = res
    return y
